# revision 16
# baseline (speedup 1.0000x reference)
"""Trainium2 Bass kernel for nn_DIFLayer (deep invertible flow layer).

Math (per row n of x, K=64 mixture components, P=64 dims, H1=H2=256):
    z_k = (x - m_k) * exp(-log_s_k)
    ref_lp_k = -0.5*||z_k||^2 - 0.5*P*log(2pi)
    h1 = tanh(W1 z_k + b1); h2 = tanh(W2 h1 + b2); logits = W3 h2 + b3
    lv_k = ref_lp_k + log_softmax(logits)[k] + logdet_k
    out = logsumexp_k(lv_k)

Device restructure:
    mm1 folds the flow into per-component weights A_k = W1*diag(inv_s_k)
    with an augmented bias contraction row (bf16).  tanh1/tanh2 are split
    between the ACT engine (exact) and a custom single-instruction DVE op
    (clamped odd deg-5 polynomial, max abs err 1.4e-2), both writing fp8e4
    directly.  mm2/mm3 run in fp8e4 with DoubleRow perf mode (2 contraction
    rows per PE cell).  ref_lp via q = U.x^2 + V.x (fp32 matmuls),
    E = exp(-0.5q + B_k).  Softmax handled unnormalized: expl = exp(logits
    + b3); per-pair selector matmuls extract S_k = sum_c expl and the diag
    D_k into a shared PSUM collector (4 pairs per bank), batch-copied to
    SBUF and row-scattered by DMA.  out = log(sum_k E*D/S) - C  (log on
    host).

Sharded data-parallel over rows: 8 cores x 2048 rows.
"""

import numpy as np

import concourse.bacc as bacc
import concourse.bass as bass
import concourse.mybir as mybir
import concourse.tile as tile
from concourse import bass_utils

# --------------------------------------------------------------------------
# Custom DVE op: tanh(u) ~= uc*(a + v*(b + v*c)), v = uc^2, uc = clamp(u,+-B)
# One DVE instruction per tile (8 ALU stages), runs concurrently with the
# ACT engine's exact tanh.  Registered at import time.
# --------------------------------------------------------------------------
from concourse.dve_spec import (
    Spec, Src0, C0, C1, C2, C3, Zero, maxx, minn, sq, _spill_c3_to_src1,
)
import concourse.dve_ops as _dve_ops_mod
from concourse.dve_ops import DveOp

TANH_B = 2.1350599
TANH_A = 0.94666379
TANH_CB = -0.19501118
TANH_CC = 0.01945195


def _tanh_pc_ref(in0, in1, s0, s1, imm2):
    prt = in0.shape[0]
    ucv = np.clip(
        in0.astype(np.float32),
        -np.asarray(s0, np.float32),
        np.asarray(s0, np.float32),
    )
    vv = ucv * ucv
    a = np.asarray(in1, np.float32).reshape(prt, -1)[:, :1]
    return ((vv * imm2 + s1) * vv + a) * ucv


_uc = maxx(minn(Src0, C0), Zero - C0)
_v = sq(_uc)
_body = _spill_c3_to_src1(((_v * C2 + C1) * _v + C3) * _uc)
TANH_PC = DveOp(
    "TANH_PC",
    Spec(body=_body, reference=_tanh_pc_ref),
    subdim=False,
    uops_sha={"v3": "b46f8204b307e3bf", "v4": "e95adf23d01b2e24"},
)
if TANH_PC.name not in _dve_ops_mod._SUB_OPCODE_FOR_NAME:
    _dve_ops_mod.OPS.append(TANH_PC)
    _dve_ops_mod._SUB_OPCODE_FOR_NAME[TANH_PC.name] = (
        _dve_ops_mod._CUSTOM_DVE_ROW_BASE + len(_dve_ops_mod.OPS) - 1
    )
    _dve_ops_mod.CUSTOM_DVE_SPECS[TANH_PC.name] = TANH_PC.spec

F32 = mybir.dt.float32
BF16 = mybir.dt.bfloat16
F8 = mybir.dt.float8e4
AFT = mybir.ActivationFunctionType
DR = mybir.MatmulPerfMode.DoubleRow

N, K, P = 16384, 64, 64
H1, H2 = 256, 256
NCORES = 8
RPC = N // NCORES          # rows per core = 2048
NT = 512                   # rows per n-tile (free dim)
TILES = RPC // NT          # 4
NPAIR = K // 2             # 32 component pairs
NGRP = NPAIR // 2          # 16 groups of 2 pairs (4 components)
LOG2PI = float(np.log(2.0 * np.pi))
C_OFF = 115.0              # global exp offset; keeps exp(lv + C) in fp32 range

_cached = {}
TRACE = False          # set by test harness to capture an NTFF profile
LAST_RESULT = None     # BassKernelResults of the most recent run


def _build_program(use_b2: bool):
    nc = bacc.Bacc("TRN2", target_bir_lowering=False, debug=False)

    xT = nc.dram_tensor("xT", [P + 1, RPC], F32, kind="ExternalInput")
    xsqT = nc.dram_tensor("xsqT", [P, RPC], F32, kind="ExternalInput")
    A_all = nc.dram_tensor("A_all", [P + 1, K * H1], BF16, kind="ExternalInput")
    W2DR = nc.dram_tensor("W2DR", [128, 2, 256], F8, kind="ExternalInput")
    W3DR = nc.dram_tensor("W3DR", [128, 2, NPAIR * 256], F8, kind="ExternalInput")
    UV = nc.dram_tensor("UV", [P, 2 * K], F32, kind="ExternalInput")
    BEx = nc.dram_tensor("BEx", [K, 1], F32, kind="ExternalInput")
    B3R = nc.dram_tensor("B3R", [128, NPAIR], F32, kind="ExternalInput")
    B2h = nc.dram_tensor("B2h", [128, 2], F32, kind="ExternalInput")
    Sel = nc.dram_tensor("Sel", [128, 2], BF16, kind="ExternalInput")
    ones = nc.dram_tensor("ones", [K, 1], F32, kind="ExternalInput")
    acc_out = nc.dram_tensor("acc_out", [1, RPC], F32, kind="ExternalOutput")

    with tile.TileContext(nc) as tc:
        with (
            tc.tile_pool(name="const", bufs=1) as cpool,
            tc.tile_pool(name="io", bufs=2) as iop,
            tc.tile_pool(name="h1pool", bufs=3) as h1pool,
            tc.tile_pool(name="h2pool", bufs=3) as h2pool,
            tc.tile_pool(name="expl", bufs=3) as explp,
            tc.tile_pool(name="ed", bufs=2) as edp,
            tc.tile_pool(name="rv", bufs=2) as rvp,
            tc.tile_pool(name="tmp", bufs=2) as tmpp,
            tc.tile_pool(name="pmlp", bufs=3, space="PSUM") as pmlp,
            tc.tile_pool(name="plg", bufs=1, space="PSUM") as plg,
            tc.tile_pool(name="pcoll", bufs=1, space="PSUM") as pcoll,
        ):
            # --- constants; small hot ones first so tile 0 can start ---
            UV_sb = cpool.tile([P, 2 * K], F32)
            nc.sync.dma_start(UV_sb[:], UV[:])
            BEx_sb = cpool.tile([K, 1], F32)
            nc.sync.dma_start(BEx_sb[:], BEx[:])
            A_sb = cpool.tile([P + 1, K * H1], BF16)
            ACH = K * H1 // 8
            W2_sb = cpool.tile([128, 2, 256], F8)
            W3_sb = cpool.tile([128, 2, NPAIR * 256], F8)
            B3r_sb = cpool.tile([128, NPAIR], F32)
            B2_sb = cpool.tile([128, 2], F32)
            ones_sb = cpool.tile([K, 1], F32)
            Sel_sb = cpool.tile([128, 2], BF16)
            aconst = cpool.tile([128, 1], F32)
            nc.vector.memset(aconst[:], TANH_A)

            gp = nc.gpsimd if hasattr(nc.gpsimd, "tensor_copy") else nc.vector

            def tanh_dve(dst, src):
                nc.vector._custom_dve(
                    TANH_PC, out=dst, in0=src, in1=aconst[:],
                    s0=TANH_B, s1=TANH_CB, imm2=TANH_CC,
                )

            def tanh_act(dst, src):
                nc.scalar.activation(dst, src, AFT.Tanh)

            NP = TILES * NPAIR          # 128 pairs, software-pipelined flat
            h1s_of, h2s_of, lg_of, ex_of = {}, {}, {}, {}
            tile_io = {}                # t -> (xt_bf, E_sb, SD_S, SD_D)
            pend_epi = [None]           # (E_sb, SD_S, SD_D, col)
            pend_acc = [None]           # (tt2, col)

            def prologue(t):
                """Input DMAs + cast + q matmuls + E exp for tile t."""
                col = slice(t * NT, (t + 1) * NT)
                xt = iop.tile([P + 1, NT], F32, tag="xt")
                nc.sync.dma_start(xt[:], xT[:, col])
                xs = iop.tile([P, NT], F32, tag="xs")
                nc.sync.dma_start(xs[:], xsqT[:, col])
                xt_bf = iop.tile([P + 1, NT], BF16, tag="xtb")
                gp.tensor_copy(xt_bf[:], xt[:])

                if t == 0:
                    def a_chunk(ch):
                        nc.sync.dma_start(
                            A_sb[:, ch * ACH : (ch + 1) * ACH],
                            A_all[:, ch * ACH : (ch + 1) * ACH],
                        )

                    WCH = NPAIR * 256 // 4
                    a_chunk(0)
                    nc.sync.dma_start(W2_sb[:], W2DR[:])
                    nc.sync.dma_start(W3_sb[:, :, 0:WCH], W3DR[:, :, 0:WCH])
                    nc.sync.dma_start(B3r_sb[:], B3R[:])
                    a_chunk(1)
                    nc.sync.dma_start(Sel_sb[:], Sel[:])
                    for ch in range(2, 8):
                        a_chunk(ch)
                        if ch <= 4:
                            w0 = (ch - 1) * WCH
                            nc.sync.dma_start(
                                W3_sb[:, :, w0 : w0 + WCH],
                                W3DR[:, :, w0 : w0 + WCH],
                            )
                    nc.sync.dma_start(B2_sb[:], B2h[:])
                    nc.sync.dma_start(ones_sb[:], ones[:])

                # E = exp(-0.5*(U.x^2 + V.x) + B_k); q borrows an lg-ring bank
                q_ps = plg.tile([128, NT], F32, tag="lg")
                nc.tensor.matmul(
                    q_ps[0:K, :], UV_sb[:, 0:K], xs[:], start=True, stop=False
                )
                nc.tensor.matmul(
                    q_ps[0:K, :], UV_sb[:, K : 2 * K], xt[0:P, :],
                    start=False, stop=True,
                )
                E_sb = edp.tile([K, NT], F32, tag="E")
                nc.scalar.activation(
                    E_sb[:], q_ps[0:K, :], AFT.Exp, bias=BEx_sb[:], scale=-0.5
                )
                SD_S = edp.tile([K, NT], F32, tag="SDs")
                SD_D = edp.tile([K, NT], BF16, tag="SDd")
                tile_io[t] = (xt_bf, E_sb, SD_S, SD_D)
                return col

            col_of = {0: prologue(0)}

            def stage_mm1_tanh1(p):
                """mm1 (bf16, augmented bias row) + tanh1 for both comps."""
                xt_bf = tile_io[p // NPAIR][0]
                for par in range(2):
                    k = 2 * (p % NPAIR) + par
                    h1p = pmlp.tile([128, 2, NT], F32, tag="mlp")
                    for half in range(2):
                        nc.tensor.matmul(
                            h1p[:, half, :],
                            A_sb[:, k * H1 + half * 128 : k * H1 + (half + 1) * 128],
                            xt_bf[:],
                            start=True,
                            stop=True,
                        )
                    h1s = h1pool.tile([128, 2, NT], F8, tag="h1s")
                    if par == 0:
                        tanh_act(h1s[:, :, :], h1p[:, :, :])
                    else:
                        tanh_dve(h1s[:, :, :], h1p[:, :, :])
                    h1s_of[(p, par)] = h1s

            def stage_mm2_tanh2(p):
                """fp8 DoubleRow mm2 + tanh2 for both comps."""
                for par in range(2):
                    h1s = h1s_of.pop((p, par))
                    h2p = pmlp.tile([128, 2, NT], F32, tag="mlp")
                    for v in range(2):
                        nc.tensor.matmul(
                            h2p[:, v, :],
                            W2_sb[:, :, v * 128 : (v + 1) * 128],
                            h1s[:, :, :],
                            start=True,
                            stop=True,
                            perf_mode=DR,
                        )
                    h2s = h2pool.tile([128, 2, NT], F8, tag="h2s")
                    if use_b2:
                        for v in range(2):
                            nc.scalar.activation(
                                h2s[:, v, :], h2p[:, v, :], AFT.Tanh,
                                bias=B2_sb[:, v : v + 1],
                            )
                    elif par == 0:
                        tanh_dve(h2s[:, :, :], h2p[:, :, :])
                    else:
                        tanh_act(h2s[:, :, :], h2p[:, :, :])
                    h2s_of[(p, par)] = h2s

            def stage_mm3_exp(p):
                """fp8 DR logits + exp.  W3 is rotated per component so the
                softmax diagonal lands at partition 0 (even) / 64 (odd), and
                padded per parity so the pair accumulates into one full-height
                PSUM bank (DR requires out base partition 0)."""
                pg = p % NPAIR
                lg2 = plg.tile([128, NT], F32, tag="lg")
                nc.tensor.matmul(
                    lg2[:], W3_sb[:, :, pg * 256 : pg * 256 + 128],
                    h2s_of.pop((p, 0))[:, :, :],
                    start=True, stop=False, perf_mode=DR,
                )
                nc.tensor.matmul(
                    lg2[:], W3_sb[:, :, pg * 256 + 128 : pg * 256 + 256],
                    h2s_of.pop((p, 1))[:, :, :],
                    start=False, stop=True, perf_mode=DR,
                )
                ex2 = explp.tile([128, NT], BF16, tag="ex")
                nc.scalar.activation(
                    ex2[:], lg2[:], AFT.Exp, bias=B3r_sb[:, pg : pg + 1]
                )
                sdd = tile_io[p // NPAIR][3]
                nc.sync.dma_start(sdd[2 * pg : 2 * pg + 1, :], ex2[0:1, :])
                nc.sync.dma_start(sdd[2 * pg + 1 : 2 * pg + 2, :], ex2[64:65, :])
                ex_of[p] = ex2

            coll_of = [None]     # current collector psum tile

            def stage_sel(p):
                """S selector matmul into a 3-pair PSUM collector (out base
                partition limited to 0/32/64); on the last block, reciprocal
                is applied directly to the collector and the 1/S rows are
                scattered into sinv64 by DMA."""
                ex2 = ex_of.pop(p)
                pg = p % NPAIR
                c, b = pg // 3, pg % 3
                if b == 0:
                    coll_new = pcoll.tile([128, NT], F32, tag="coll")
                    coll_of[0] = coll_new
                coll = coll_of[0]
                nc.tensor.matmul(
                    coll[32 * b : 32 * b + 2, :], Sel_sb[:], ex2[:],
                    start=True, stop=True,
                )
                last = pg == NPAIR - 1
                if b == 2 or last:
                    rinv = rvp.tile([128, NT], F32, tag="rv")
                    hi = 32 * b + 2
                    nc.vector.reciprocal(rinv[0:hi, :], coll[0:hi, :])
                    sinv = tile_io[p // NPAIR][2]
                    for bb in range(b + 1):
                        nc.sync.dma_start(
                            sinv[6 * c + 2 * bb : 6 * c + 2 * bb + 2, :],
                            rinv[32 * bb : 32 * bb + 2, :],
                        )

            def flush_epi_dve():
                if pend_epi[0] is None:
                    return
                E_prev, sinv, sdd, pcol = pend_epi[0]
                pend_epi[0] = None
                tt = tmpp.tile([K, NT], F32, tag="tmp2")
                gp.tensor_mul(tt[:], sdd[:], sinv[:])
                tt2 = tmpp.tile([K, NT], F32, tag="tt2")
                gp.tensor_mul(tt2[:], tt[:], E_prev[:])
                pend_acc[0] = (tt2, pend_col[0])

            def flush_acc():
                if pend_acc[0] is None:
                    return
                tt2, acc_col = pend_acc[0]
                pend_acc[0] = None
                acc_ps = plg.tile([128, NT], F32, tag="lg")
                nc.tensor.matmul(
                    acc_ps[0:1, :], ones_sb[:], tt2[:], start=True, stop=True
                )
                acc_sb = tmpp.tile([1, NT], F32, tag="acc")
                nc.vector.tensor_copy(acc_sb[:], acc_ps[0:1, :])
                nc.sync.dma_start(acc_out[0:1, acc_col], acc_sb[:])

            pend_col = [None]

            for it in range(NP + 3):
                if it >= 3:
                    stage_sel(it - 3)
                if it % NPAIR == 16:
                    flush_acc()
                if 1 <= it <= NP:
                    stage_mm2_tanh2(it - 1)
                if it < NP:
                    stage_mm1_tanh1(it)
                if 2 <= it <= NP + 1:
                    stage_mm3_exp(it - 2)
                if it < NP and it % NPAIR == NPAIR - 4:
                    t = it // NPAIR
                    if t + 1 < TILES:
                        col_of[t + 1] = prologue(t + 1)
                if it % NPAIR == NPAIR - 1:
                    t = it // NPAIR
                    _, E_sb, SD_S, SD_D = tile_io[t]
                    pend_epi[0] = (E_sb, SD_S, SD_D, col_of[t])
                if it % NPAIR == 8 and it > NPAIR:
                    pend_col[0] = col_of[it // NPAIR - 1]
                    flush_epi_dve()

            # tail: epilogue for the last tile
            pend_col[0] = col_of[TILES - 1]
            flush_epi_dve()
            flush_acc()

    nc.finalize()
    return nc


def _prep_consts(m, log_s, W1, b1, W2, b2, W3, b3):
    import ml_dtypes

    bf16 = ml_dtypes.bfloat16
    f8 = ml_dtypes.float8_e4m3fn
    inv_s = np.exp(-np.asarray(log_s, np.float64))          # [K,P]
    m64 = np.asarray(m, np.float64)
    W1_64 = np.asarray(W1, np.float64)
    ims = inv_s * m64                                       # [K,P]

    # A_all[p, k*H1+h] = W1[h,p]*inv_s[k,p]; row P = c1_k[h]
    A = W1_64[None, :, :] * inv_s[:, None, :]               # [K,H1,P]
    A_all = np.empty((P + 1, K * H1), np.float32)
    A_all[:P] = A.transpose(2, 0, 1).reshape(P, K * H1)
    c1 = np.asarray(b1, np.float64)[None, :] - np.einsum("hp,kp->kh", W1_64, ims)
    A_all[P] = c1.reshape(K * H1).astype(np.float32)

    # DoubleRow fp8 weights: lhsT[p, j, o] = W[o, p + 128j]
    W2a = np.asarray(W2, np.float32)
    W2dr = np.empty((128, 2, 256), np.float32)
    for j in range(2):
        for v in range(2):
            W2dr[:, j, v * 128 : (v + 1) * 128] = W2a[
                v * 128 : (v + 1) * 128, 128 * j : 128 * (j + 1)
            ].T
    # W3 per-pair stationaries: rotated per component so comp k's diagonal
    # class lands at out partition 0 (even) / 64 (odd), padded to 128 out
    # columns per parity (DR requires out base partition 0; the pair
    # accumulates into one full-height PSUM bank).
    W3a = np.asarray(W3, np.float32)
    W3dr = np.zeros((128, 2, NPAIR * 256), np.float32)
    cidx = np.arange(64)
    for pg in range(NPAIR):
        for par in range(2):
            k = 2 * pg + par
            rot = W3a[(cidx + k) % 64, :]          # [c', 256]
            for j in range(2):
                base = pg * 256 + 192 * par
                W3dr[:, j, base : base + 64] = rot[:, 128 * j : 128 * (j + 1)].T

    UV = np.empty((P, 2 * K), np.float32)
    UV[:, 0:K] = (inv_s**2).T
    UV[:, K : 2 * K] = (-2.0 * m64 * inv_s**2).T

    w_k = np.sum(ims**2, axis=1)                            # [K]
    log_det = -np.asarray(log_s, np.float64).sum(axis=1)    # [K]
    BEx = (-0.5 * w_k - 0.5 * P * LOG2PI + log_det + C_OFF).astype(np.float32)

    b3a = np.asarray(b3, np.float32)
    B3R = np.empty((128, NPAIR), np.float32)
    for pg in range(NPAIR):
        B3R[0:64, pg] = b3a[(cidx + 2 * pg) % 64]
        B3R[64:128, pg] = b3a[(cidx + 2 * pg + 1) % 64]
    B2h = np.stack([np.asarray(b2)[:128], np.asarray(b2)[128:]], axis=1).astype(
        np.float32
    )

    Sel = np.zeros((128, 2), np.float32)
    Sel[0:64, 0] = 1.0
    Sel[64:128, 1] = 1.0

    return {
        "A_all": A_all.astype(bf16),
        "W2DR": W2dr.astype(f8),
        "W3DR": W3dr.astype(f8),
        "UV": UV,
        "BEx": BEx.reshape(K, 1),
        "B3R": B3R,
        "B2h": B2h,
        "Sel": Sel.astype(bf16),
        "ones": np.ones((K, 1), np.float32),
    }


def kernel(x, m, log_s, W1, b1, W2, b2, W3, b3):
    x = np.asarray(x, np.float32)
    consts = _prep_consts(m, log_s, W1, b1, W2, b2, W3, b3)
    use_b2 = bool(np.any(np.asarray(b2)))

    key = ("prog", use_b2)
    if key not in _cached:
        _cached[key] = _build_program(use_b2)
    nc = _cached[key]

    xT = np.empty((P + 1, N), np.float32)
    xT[:P] = x.T
    xT[P] = 1.0
    xsqT = (x.T.astype(np.float64) ** 2).astype(np.float32)

    in_maps = []
    for i in range(NCORES):
        col = slice(i * RPC, (i + 1) * RPC)
        im = {"xT": np.ascontiguousarray(xT[:, col]),
              "xsqT": np.ascontiguousarray(xsqT[:, col])}
        im.update(consts)
        in_maps.append(im)

    res = bass_utils.run_bass_kernel_spmd(
        nc, in_maps, list(range(NCORES)), trace=TRACE
    )
    global LAST_RESULT
    LAST_RESULT = res
    acc = np.concatenate([r["acc_out"].reshape(RPC) for r in res.results])
    return (np.log(acc.astype(np.float64)) - C_OFF).astype(np.float32)


# revision 17
# speedup vs baseline: 1.4936x; 1.4936x over previous
"""Trainium2 Bass kernel for nn_DIFLayer (deep invertible flow layer).

Math (per row n of x, K=64 mixture components, P=64 dims, H1=H2=256):
    z_k = (x - m_k) * exp(-log_s_k)
    ref_lp_k = -0.5*||z_k||^2 - 0.5*P*log(2pi)
    h1 = tanh(W1 z_k + b1); h2 = tanh(W2 h1 + b2); logits = W3 h2 + b3
    lv_k = ref_lp_k + log_softmax(logits)[k] + logdet_k
    out = logsumexp_k(lv_k)

Device restructure:
    mm1 folds the flow into per-component weights A_k = W1*diag(inv_s_k)
    with an augmented bias contraction row (bf16).  tanh1/tanh2 are split
    between the ACT engine (exact) and a custom single-instruction DVE op
    (clamped odd deg-5 polynomial, max abs err 1.4e-2), both writing fp8e4
    directly.  mm2/mm3 run in fp8e4 with DoubleRow perf mode (2 contraction
    rows per PE cell).  ref_lp via q = U.x^2 + V.x (fp32 matmuls),
    E = exp(-0.5q + B_k).  Softmax handled unnormalized: expl = exp(logits
    + b3); per-pair selector matmuls extract S_k = sum_c expl and the diag
    D_k into a shared PSUM collector (4 pairs per bank), batch-copied to
    SBUF and row-scattered by DMA.  out = log(sum_k E*D/S) - C  (log on
    host).

Sharded data-parallel over rows: 8 cores x 2048 rows.
"""

import numpy as np

import concourse.bacc as bacc
import concourse.bass as bass
import concourse.mybir as mybir
import concourse.tile as tile
from concourse import bass_utils

# --------------------------------------------------------------------------
# Custom DVE op: tanh(u) ~= uc*(a + v*(b + v*c)), v = uc^2, uc = clamp(u,+-B)
# One DVE instruction per tile (8 ALU stages), runs concurrently with the
# ACT engine's exact tanh.  Registered at import time.
# --------------------------------------------------------------------------
from concourse.dve_spec import (
    Spec, Src0, C0, C1, C2, C3, Zero, maxx, minn, sq, _spill_c3_to_src1,
)
import concourse.dve_ops as _dve_ops_mod
from concourse.dve_ops import DveOp

TANH_B = 2.1350599
TANH_A = 0.94666379
TANH_CB = -0.19501118
TANH_CC = 0.01945195


def _tanh_pc_ref(in0, in1, s0, s1, imm2):
    prt = in0.shape[0]
    ucv = np.clip(
        in0.astype(np.float32),
        -np.asarray(s0, np.float32),
        np.asarray(s0, np.float32),
    )
    vv = ucv * ucv
    a = np.asarray(in1, np.float32).reshape(prt, -1)[:, :1]
    return ((vv * imm2 + s1) * vv + a) * ucv


_uc = maxx(minn(Src0, C0), Zero - C0)
_v = sq(_uc)
_body = _spill_c3_to_src1(((_v * C2 + C1) * _v + C3) * _uc)
TANH_PC = DveOp(
    "TANH_PC",
    Spec(body=_body, reference=_tanh_pc_ref),
    subdim=False,
    uops_sha={"v3": "b46f8204b307e3bf", "v4": "e95adf23d01b2e24"},
)
if TANH_PC.name not in _dve_ops_mod._SUB_OPCODE_FOR_NAME:
    _dve_ops_mod.OPS.append(TANH_PC)
    _dve_ops_mod._SUB_OPCODE_FOR_NAME[TANH_PC.name] = (
        _dve_ops_mod._CUSTOM_DVE_ROW_BASE + len(_dve_ops_mod.OPS) - 1
    )
    _dve_ops_mod.CUSTOM_DVE_SPECS[TANH_PC.name] = TANH_PC.spec

F32 = mybir.dt.float32
BF16 = mybir.dt.bfloat16
F8 = mybir.dt.float8e4
AFT = mybir.ActivationFunctionType
DR = mybir.MatmulPerfMode.DoubleRow

N, K, P = 16384, 64, 64
H1, H2 = 256, 256
NCORES = 8
RPC = N // NCORES          # rows per core = 2048
NT = 512                   # rows per n-tile (free dim)
TILES = RPC // NT          # 4
NPAIR = K // 2             # 32 component pairs
NGRP = NPAIR // 2          # 16 groups of 2 pairs (4 components)
LOG2PI = float(np.log(2.0 * np.pi))
C_OFF = 115.0              # global exp offset; keeps exp(lv + C) in fp32 range

_cached = {}
TRACE = False          # set by test harness to capture an NTFF profile
LAST_RESULT = None     # BassKernelResults of the most recent run


def _build_program(use_b2: bool):
    nc = bacc.Bacc("TRN2", target_bir_lowering=False, debug=False)

    xT = nc.dram_tensor("xT", [P + 1, RPC], F32, kind="ExternalInput")
    xsqT = nc.dram_tensor("xsqT", [P, RPC], F32, kind="ExternalInput")
    A_all = nc.dram_tensor("A_all", [P + 1, K * H1], BF16, kind="ExternalInput")
    W2DR = nc.dram_tensor("W2DR", [128, 2, 256], F8, kind="ExternalInput")
    W3DR = nc.dram_tensor("W3DR", [128, 2, NPAIR * 256], F8, kind="ExternalInput")
    UV = nc.dram_tensor("UV", [P, 2 * K], F32, kind="ExternalInput")
    BEx = nc.dram_tensor("BEx", [K, 1], F32, kind="ExternalInput")
    B3R = nc.dram_tensor("B3R", [128, NPAIR], F32, kind="ExternalInput")
    B2h = nc.dram_tensor("B2h", [128, 2], F32, kind="ExternalInput")
    Sel = nc.dram_tensor("Sel", [128, 2], BF16, kind="ExternalInput")
    ones = nc.dram_tensor("ones", [K, 1], F32, kind="ExternalInput")
    acc_out = nc.dram_tensor("acc_out", [1, RPC], F32, kind="ExternalOutput")

    with tile.TileContext(nc) as tc:
        with (
            tc.tile_pool(name="const", bufs=1) as cpool,
            tc.tile_pool(name="io", bufs=2) as iop,
            tc.tile_pool(name="h1pool", bufs=3) as h1pool,
            tc.tile_pool(name="h2pool", bufs=3) as h2pool,
            tc.tile_pool(name="expl", bufs=3) as explp,
            tc.tile_pool(name="ed", bufs=2) as edp,
            tc.tile_pool(name="rv", bufs=2) as rvp,
            tc.tile_pool(name="tmp", bufs=2) as tmpp,
            tc.tile_pool(name="pmlp", bufs=3, space="PSUM") as pmlp,
            tc.tile_pool(name="plg", bufs=1, space="PSUM") as plg,
            tc.tile_pool(name="pcoll", bufs=1, space="PSUM") as pcoll,
        ):
            # --- constants; small hot ones first so tile 0 can start ---
            UV_sb = cpool.tile([P, 2 * K], F32)
            nc.sync.dma_start(UV_sb[:], UV[:])
            BEx_sb = cpool.tile([K, 1], F32)
            nc.sync.dma_start(BEx_sb[:], BEx[:])
            A_sb = cpool.tile([P + 1, K * H1], BF16)
            ACH = K * H1 // 8
            W2_sb = cpool.tile([128, 2, 256], F8)
            W3_sb = cpool.tile([128, 2, NPAIR * 256], F8)
            B3r_sb = cpool.tile([128, NPAIR], F32)
            B2_sb = cpool.tile([128, 2], F32)
            ones_sb = cpool.tile([K, 1], F32)
            Sel_sb = cpool.tile([128, 2], BF16)
            aconst = cpool.tile([128, 1], F32)
            nc.vector.memset(aconst[:], TANH_A)

            gp = nc.gpsimd if hasattr(nc.gpsimd, "tensor_copy") else nc.vector

            def tanh_dve(dst, src):
                nc.vector._custom_dve(
                    TANH_PC, out=dst, in0=src, in1=aconst[:],
                    s0=TANH_B, s1=TANH_CB, imm2=TANH_CC,
                )

            def tanh_act(dst, src):
                nc.scalar.activation(dst, src, AFT.Tanh)

            NP = TILES * NPAIR          # 128 pairs, software-pipelined flat
            h1s_of, h2s_of, lg_of, ex_of = {}, {}, {}, {}
            tile_io = {}                # t -> (xt_bf, E_sb, SD_S, SD_D)
            pend_epi = [None]           # (E_sb, SD_S, SD_D, col)
            pend_acc = [None]           # (tt2, col)

            def prologue(t):
                """Input DMAs + cast + q matmuls + E exp for tile t."""
                col = slice(t * NT, (t + 1) * NT)
                xt = iop.tile([P + 1, NT], F32, tag="xt")
                nc.sync.dma_start(xt[:], xT[:, col])
                xs = iop.tile([P, NT], F32, tag="xs")
                nc.sync.dma_start(xs[:], xsqT[:, col])
                xt_bf = iop.tile([P + 1, NT], BF16, tag="xtb")
                gp.tensor_copy(xt_bf[:], xt[:])

                if t == 0:
                    def a_chunk(ch):
                        nc.sync.dma_start(
                            A_sb[:, ch * ACH : (ch + 1) * ACH],
                            A_all[:, ch * ACH : (ch + 1) * ACH],
                        )

                    WCH = NPAIR * 256 // 4
                    a_chunk(0)
                    nc.sync.dma_start(W2_sb[:], W2DR[:])
                    nc.sync.dma_start(W3_sb[:, :, 0:WCH], W3DR[:, :, 0:WCH])
                    nc.sync.dma_start(B3r_sb[:], B3R[:])
                    a_chunk(1)
                    nc.sync.dma_start(Sel_sb[:], Sel[:])
                    for ch in range(2, 8):
                        a_chunk(ch)
                        if ch <= 4:
                            w0 = (ch - 1) * WCH
                            nc.sync.dma_start(
                                W3_sb[:, :, w0 : w0 + WCH],
                                W3DR[:, :, w0 : w0 + WCH],
                            )
                    nc.sync.dma_start(B2_sb[:], B2h[:])
                    nc.sync.dma_start(ones_sb[:], ones[:])

                # E = exp(-0.5*(U.x^2 + V.x) + B_k); q borrows an lg-ring bank
                q_ps = plg.tile([128, NT], F32, tag="lg")
                nc.tensor.matmul(
                    q_ps[0:K, :], UV_sb[:, 0:K], xs[:], start=True, stop=False
                )
                nc.tensor.matmul(
                    q_ps[0:K, :], UV_sb[:, K : 2 * K], xt[0:P, :],
                    start=False, stop=True,
                )
                E_sb = edp.tile([K, NT], F32, tag="E")
                nc.scalar.activation(
                    E_sb[:], q_ps[0:K, :], AFT.Exp, bias=BEx_sb[:], scale=-0.5
                )
                SD_S = edp.tile([K, NT], F32, tag="SDs")
                SD_D = edp.tile([K, NT], BF16, tag="SDd")
                tile_io[t] = (xt_bf, E_sb, SD_S, SD_D)
                return col

            col_of = {0: prologue(0)}

            def stage_mm1_tanh1(p):
                """mm1 (bf16, augmented bias row) + tanh1 for both comps."""
                xt_bf = tile_io[p // NPAIR][0]
                for par in range(2):
                    k = 2 * (p % NPAIR) + par
                    h1p = pmlp.tile([128, 2, NT], F32, tag="mlp")
                    for half in range(2):
                        nc.tensor.matmul(
                            h1p[:, half, :],
                            A_sb[:, k * H1 + half * 128 : k * H1 + (half + 1) * 128],
                            xt_bf[:],
                            start=True,
                            stop=True,
                        )
                    h1s = h1pool.tile([128, 2, NT], F8, tag="h1s")
                    if par == 0:
                        tanh_act(h1s[:, :, :], h1p[:, :, :])
                    else:
                        tanh_dve(h1s[:, :, :], h1p[:, :, :])
                    h1s_of[(p, par)] = h1s

            def stage_mm2_tanh2(p):
                """fp8 DoubleRow mm2 + tanh2 for both comps."""
                for par in range(2):
                    h1s = h1s_of.pop((p, par))
                    h2p = pmlp.tile([128, 2, NT], F32, tag="mlp")
                    for v in range(2):
                        nc.tensor.matmul(
                            h2p[:, v, :],
                            W2_sb[:, :, v * 128 : (v + 1) * 128],
                            h1s[:, :, :],
                            start=True,
                            stop=True,
                            perf_mode=DR,
                        )
                    h2s = h2pool.tile([128, 2, NT], F8, tag="h2s")
                    if use_b2:
                        for v in range(2):
                            nc.scalar.activation(
                                h2s[:, v, :], h2p[:, v, :], AFT.Tanh,
                                bias=B2_sb[:, v : v + 1],
                            )
                    elif par == 0:
                        tanh_dve(h2s[:, :, :], h2p[:, :, :])
                    else:
                        tanh_act(h2s[:, :, :], h2p[:, :, :])
                    h2s_of[(p, par)] = h2s

            def stage_mm3_exp(p):
                """fp8 DR logits + exp.  W3 is rotated per component so the
                softmax diagonal lands at partition 0 (even) / 64 (odd), and
                padded per parity so the pair accumulates into one full-height
                PSUM bank (DR requires out base partition 0)."""
                pg = p % NPAIR
                lg2 = plg.tile([128, NT], F32, tag="lg")
                nc.tensor.matmul(
                    lg2[:], W3_sb[:, :, pg * 256 : pg * 256 + 128],
                    h2s_of.pop((p, 0))[:, :, :],
                    start=True, stop=False, perf_mode=DR,
                )
                nc.tensor.matmul(
                    lg2[:], W3_sb[:, :, pg * 256 + 128 : pg * 256 + 256],
                    h2s_of.pop((p, 1))[:, :, :],
                    start=False, stop=True, perf_mode=DR,
                )
                ex2 = explp.tile([128, NT], BF16, tag="ex")
                nc.scalar.activation(
                    ex2[:], lg2[:], AFT.Exp, bias=B3r_sb[:, pg : pg + 1]
                )
                sdd = tile_io[p // NPAIR][3]
                nc.sync.dma_start(sdd[2 * pg : 2 * pg + 1, :], ex2[0:1, :])
                nc.sync.dma_start(sdd[2 * pg + 1 : 2 * pg + 2, :], ex2[64:65, :])
                ex_of[p] = ex2

            coll_of = [None]     # current collector psum tile

            def stage_sel(p):
                """S selector matmul into a 3-pair PSUM collector (out base
                partition limited to 0/32/64); on the last block, reciprocal
                is applied directly to the collector and the 1/S rows are
                scattered into sinv64 by DMA."""
                ex2 = ex_of.pop(p)
                pg = p % NPAIR
                c, b = pg // 3, pg % 3
                if b == 0:
                    coll_new = pcoll.tile([128, NT], F32, tag="coll")
                    coll_of[0] = coll_new
                coll = coll_of[0]
                nc.tensor.matmul(
                    coll[32 * b : 32 * b + 2, :], Sel_sb[:], ex2[:],
                    start=True, stop=True,
                )
                last = pg == NPAIR - 1
                if b == 2 or last:
                    rinv = rvp.tile([128, NT], F32, tag="rv")
                    hi = 32 * b + 2
                    nc.vector.reciprocal_approx_fast(rinv[0:hi, :], coll[0:hi, :])
                    sinv = tile_io[p // NPAIR][2]
                    for bb in range(b + 1):
                        nc.sync.dma_start(
                            sinv[6 * c + 2 * bb : 6 * c + 2 * bb + 2, :],
                            rinv[32 * bb : 32 * bb + 2, :],
                        )

            def flush_epi_dve():
                if pend_epi[0] is None:
                    return
                E_prev, sinv, sdd, pcol = pend_epi[0]
                pend_epi[0] = None
                tt = tmpp.tile([K, NT], F32, tag="tmp2")
                gp.tensor_mul(tt[:], sdd[:], sinv[:])
                tt2 = tmpp.tile([K, NT], F32, tag="tt2")
                gp.tensor_mul(tt2[:], tt[:], E_prev[:])
                pend_acc[0] = (tt2, pend_col[0])

            def flush_acc():
                if pend_acc[0] is None:
                    return
                tt2, acc_col = pend_acc[0]
                pend_acc[0] = None
                acc_ps = plg.tile([128, NT], F32, tag="lg")
                nc.tensor.matmul(
                    acc_ps[0:1, :], ones_sb[:], tt2[:], start=True, stop=True
                )
                acc_sb = tmpp.tile([1, NT], F32, tag="acc")
                nc.vector.tensor_copy(acc_sb[:], acc_ps[0:1, :])
                nc.sync.dma_start(acc_out[0:1, acc_col], acc_sb[:])

            pend_col = [None]

            for it in range(NP + 3):
                if it >= 3:
                    stage_sel(it - 3)
                if it % NPAIR == 16:
                    flush_acc()
                if 1 <= it <= NP:
                    stage_mm2_tanh2(it - 1)
                if it < NP:
                    stage_mm1_tanh1(it)
                if 2 <= it <= NP + 1:
                    stage_mm3_exp(it - 2)
                if it < NP and it % NPAIR == NPAIR - 4:
                    t = it // NPAIR
                    if t + 1 < TILES:
                        col_of[t + 1] = prologue(t + 1)
                if it % NPAIR == NPAIR - 1:
                    t = it // NPAIR
                    _, E_sb, SD_S, SD_D = tile_io[t]
                    pend_epi[0] = (E_sb, SD_S, SD_D, col_of[t])
                if it % NPAIR == 8 and it > NPAIR:
                    pend_col[0] = col_of[it // NPAIR - 1]
                    flush_epi_dve()

            # tail: epilogue for the last tile
            pend_col[0] = col_of[TILES - 1]
            flush_epi_dve()
            flush_acc()

    nc.finalize()
    return nc


def _prep_consts(m, log_s, W1, b1, W2, b2, W3, b3):
    import ml_dtypes

    bf16 = ml_dtypes.bfloat16
    f8 = ml_dtypes.float8_e4m3fn
    inv_s = np.exp(-np.asarray(log_s, np.float64))          # [K,P]
    m64 = np.asarray(m, np.float64)
    W1_64 = np.asarray(W1, np.float64)
    ims = inv_s * m64                                       # [K,P]

    # A_all[p, k*H1+h] = W1[h,p]*inv_s[k,p]; row P = c1_k[h]
    A = W1_64[None, :, :] * inv_s[:, None, :]               # [K,H1,P]
    A_all = np.empty((P + 1, K * H1), np.float32)
    A_all[:P] = A.transpose(2, 0, 1).reshape(P, K * H1)
    c1 = np.asarray(b1, np.float64)[None, :] - np.einsum("hp,kp->kh", W1_64, ims)
    A_all[P] = c1.reshape(K * H1).astype(np.float32)

    # DoubleRow fp8 weights: lhsT[p, j, o] = W[o, p + 128j]
    W2a = np.asarray(W2, np.float32)
    W2dr = np.empty((128, 2, 256), np.float32)
    for j in range(2):
        for v in range(2):
            W2dr[:, j, v * 128 : (v + 1) * 128] = W2a[
                v * 128 : (v + 1) * 128, 128 * j : 128 * (j + 1)
            ].T
    # W3 per-pair stationaries: rotated per component so comp k's diagonal
    # class lands at out partition 0 (even) / 64 (odd), padded to 128 out
    # columns per parity (DR requires out base partition 0; the pair
    # accumulates into one full-height PSUM bank).
    W3a = np.asarray(W3, np.float32)
    W3dr = np.zeros((128, 2, NPAIR * 256), np.float32)
    cidx = np.arange(64)
    for pg in range(NPAIR):
        for par in range(2):
            k = 2 * pg + par
            rot = W3a[(cidx + k) % 64, :]          # [c', 256]
            for j in range(2):
                base = pg * 256 + 192 * par
                W3dr[:, j, base : base + 64] = rot[:, 128 * j : 128 * (j + 1)].T

    UV = np.empty((P, 2 * K), np.float32)
    UV[:, 0:K] = (inv_s**2).T
    UV[:, K : 2 * K] = (-2.0 * m64 * inv_s**2).T

    w_k = np.sum(ims**2, axis=1)                            # [K]
    log_det = -np.asarray(log_s, np.float64).sum(axis=1)    # [K]
    BEx = (-0.5 * w_k - 0.5 * P * LOG2PI + log_det + C_OFF).astype(np.float32)

    b3a = np.asarray(b3, np.float32)
    B3R = np.empty((128, NPAIR), np.float32)
    for pg in range(NPAIR):
        B3R[0:64, pg] = b3a[(cidx + 2 * pg) % 64]
        B3R[64:128, pg] = b3a[(cidx + 2 * pg + 1) % 64]
    B2h = np.stack([np.asarray(b2)[:128], np.asarray(b2)[128:]], axis=1).astype(
        np.float32
    )

    Sel = np.zeros((128, 2), np.float32)
    Sel[0:64, 0] = 1.0
    Sel[64:128, 1] = 1.0

    return {
        "A_all": A_all.astype(bf16),
        "W2DR": W2dr.astype(f8),
        "W3DR": W3dr.astype(f8),
        "UV": UV,
        "BEx": BEx.reshape(K, 1),
        "B3R": B3R,
        "B2h": B2h,
        "Sel": Sel.astype(bf16),
        "ones": np.ones((K, 1), np.float32),
    }


def kernel(x, m, log_s, W1, b1, W2, b2, W3, b3):
    x = np.asarray(x, np.float32)
    consts = _prep_consts(m, log_s, W1, b1, W2, b2, W3, b3)
    use_b2 = bool(np.any(np.asarray(b2)))

    key = ("prog", use_b2)
    if key not in _cached:
        _cached[key] = _build_program(use_b2)
    nc = _cached[key]

    xT = np.empty((P + 1, N), np.float32)
    xT[:P] = x.T
    xT[P] = 1.0
    xsqT = (x.T.astype(np.float64) ** 2).astype(np.float32)

    in_maps = []
    for i in range(NCORES):
        col = slice(i * RPC, (i + 1) * RPC)
        im = {"xT": np.ascontiguousarray(xT[:, col]),
              "xsqT": np.ascontiguousarray(xsqT[:, col])}
        im.update(consts)
        in_maps.append(im)

    res = bass_utils.run_bass_kernel_spmd(
        nc, in_maps, list(range(NCORES)), trace=TRACE
    )
    global LAST_RESULT
    LAST_RESULT = res
    acc = np.concatenate([r["acc_out"].reshape(RPC) for r in res.results])
    return (np.log(acc.astype(np.float64)) - C_OFF).astype(np.float32)


# revision 18
# speedup vs baseline: 1.5082x; 1.0098x over previous
"""Trainium2 Bass kernel for nn_DIFLayer (deep invertible flow layer).

Math (per row n of x, K=64 mixture components, P=64 dims, H1=H2=256):
    z_k = (x - m_k) * exp(-log_s_k)
    ref_lp_k = -0.5*||z_k||^2 - 0.5*P*log(2pi)
    h1 = tanh(W1 z_k + b1); h2 = tanh(W2 h1 + b2); logits = W3 h2 + b3
    lv_k = ref_lp_k + log_softmax(logits)[k] + logdet_k
    out = logsumexp_k(lv_k)

Device restructure:
    mm1 folds the flow into per-component weights A_k = W1*diag(inv_s_k)
    with an augmented bias contraction row (bf16).  tanh1/tanh2 are split
    between the ACT engine (exact) and a custom single-instruction DVE op
    (clamped odd deg-5 polynomial, max abs err 1.4e-2), both writing fp8e4
    directly.  mm2/mm3 run in fp8e4 with DoubleRow perf mode (2 contraction
    rows per PE cell).  ref_lp via q = U.x^2 + V.x (fp32 matmuls),
    E = exp(-0.5q + B_k).  Softmax handled unnormalized: expl = exp(logits
    + b3); per-pair selector matmuls extract S_k = sum_c expl and the diag
    D_k into a shared PSUM collector (4 pairs per bank), batch-copied to
    SBUF and row-scattered by DMA.  out = log(sum_k E*D/S) - C  (log on
    host).

Sharded data-parallel over rows: 8 cores x 2048 rows.
"""

import numpy as np

import concourse.bacc as bacc
import concourse.bass as bass
import concourse.mybir as mybir
import concourse.tile as tile
from concourse import bass_utils

# --------------------------------------------------------------------------
# Custom DVE op: tanh(u) ~= uc*(a + v*(b + v*c)), v = uc^2, uc = clamp(u,+-B)
# One DVE instruction per tile (8 ALU stages), runs concurrently with the
# ACT engine's exact tanh.  Registered at import time.
# --------------------------------------------------------------------------
from concourse.dve_spec import (
    Spec, Src0, C0, C1, C2, C3, Zero, maxx, minn, sq, _spill_c3_to_src1,
)
import concourse.dve_ops as _dve_ops_mod
from concourse.dve_ops import DveOp

TANH_B = 2.1350599
TANH_A = 0.94666379
TANH_CB = -0.19501118
TANH_CC = 0.01945195


def _tanh_pc_ref(in0, in1, s0, s1, imm2):
    prt = in0.shape[0]
    ucv = np.clip(
        in0.astype(np.float32),
        -np.asarray(s0, np.float32),
        np.asarray(s0, np.float32),
    )
    vv = ucv * ucv
    a = np.asarray(in1, np.float32).reshape(prt, -1)[:, :1]
    return ((vv * imm2 + s1) * vv + a) * ucv


_uc = maxx(minn(Src0, C0), Zero - C0)
_v = sq(_uc)
_body = _spill_c3_to_src1(((_v * C2 + C1) * _v + C3) * _uc)
TANH_PC = DveOp(
    "TANH_PC",
    Spec(body=_body, reference=_tanh_pc_ref),
    subdim=False,
    uops_sha={"v3": "b46f8204b307e3bf", "v4": "e95adf23d01b2e24"},
)
if TANH_PC.name not in _dve_ops_mod._SUB_OPCODE_FOR_NAME:
    _dve_ops_mod.OPS.append(TANH_PC)
    _dve_ops_mod._SUB_OPCODE_FOR_NAME[TANH_PC.name] = (
        _dve_ops_mod._CUSTOM_DVE_ROW_BASE + len(_dve_ops_mod.OPS) - 1
    )
    _dve_ops_mod.CUSTOM_DVE_SPECS[TANH_PC.name] = TANH_PC.spec

F32 = mybir.dt.float32
BF16 = mybir.dt.bfloat16
F8 = mybir.dt.float8e4
AFT = mybir.ActivationFunctionType
DR = mybir.MatmulPerfMode.DoubleRow

N, K, P = 16384, 64, 64
H1, H2 = 256, 256
NCORES = 8
RPC = N // NCORES          # rows per core = 2048
NT = 512                   # rows per n-tile (free dim)
TILES = RPC // NT          # 4
NPAIR = K // 2             # 32 component pairs
NGRP = NPAIR // 2          # 16 groups of 2 pairs (4 components)
LOG2PI = float(np.log(2.0 * np.pi))
C_OFF = 115.0              # global exp offset; keeps exp(lv + C) in fp32 range

_cached = {}
TRACE = False          # set by test harness to capture an NTFF profile
LAST_RESULT = None     # BassKernelResults of the most recent run


def _build_program(use_b2: bool):
    nc = bacc.Bacc("TRN2", target_bir_lowering=False, debug=False)

    xT = nc.dram_tensor("xT", [P + 1, RPC], F32, kind="ExternalInput")
    xsqT = nc.dram_tensor("xsqT", [P, RPC], F32, kind="ExternalInput")
    A_all = nc.dram_tensor("A_all", [P + 1, K * H1], BF16, kind="ExternalInput")
    W2DR = nc.dram_tensor("W2DR", [128, 2, 256], F8, kind="ExternalInput")
    W3DR = nc.dram_tensor("W3DR", [128, 2, NPAIR * 256], F8, kind="ExternalInput")
    UV = nc.dram_tensor("UV", [P, 2 * K], F32, kind="ExternalInput")
    BEx = nc.dram_tensor("BEx", [K, 1], F32, kind="ExternalInput")
    B3R = nc.dram_tensor("B3R", [128, NPAIR], F32, kind="ExternalInput")
    B2h = nc.dram_tensor("B2h", [128, 2], F32, kind="ExternalInput")
    Sel = nc.dram_tensor("Sel", [128, 2], BF16, kind="ExternalInput")
    ones = nc.dram_tensor("ones", [K, 1], F32, kind="ExternalInput")
    acc_out = nc.dram_tensor("acc_out", [1, RPC], F32, kind="ExternalOutput")

    with tile.TileContext(nc) as tc:
        with (
            tc.tile_pool(name="const", bufs=1) as cpool,
            tc.tile_pool(name="io", bufs=2) as iop,
            tc.tile_pool(name="h1pool", bufs=3) as h1pool,
            tc.tile_pool(name="h2pool", bufs=3) as h2pool,
            tc.tile_pool(name="expl", bufs=3) as explp,
            tc.tile_pool(name="ed", bufs=2) as edp,
            tc.tile_pool(name="rv", bufs=2) as rvp,
            tc.tile_pool(name="tmp", bufs=2) as tmpp,
            tc.tile_pool(name="pmlp", bufs=2, space="PSUM") as pmlp,
            tc.tile_pool(name="plg", bufs=2, space="PSUM") as plg,
            tc.tile_pool(name="pcoll", bufs=2, space="PSUM") as pcoll,
        ):
            # --- constants; small hot ones first so tile 0 can start ---
            UV_sb = cpool.tile([P, 2 * K], F32)
            nc.sync.dma_start(UV_sb[:], UV[:])
            BEx_sb = cpool.tile([K, 1], F32)
            nc.sync.dma_start(BEx_sb[:], BEx[:])
            A_sb = cpool.tile([P + 1, K * H1], BF16)
            ACH = K * H1 // 8
            W2_sb = cpool.tile([128, 2, 256], F8)
            W3_sb = cpool.tile([128, 2, NPAIR * 256], F8)
            B3r_sb = cpool.tile([128, NPAIR], F32)
            B2_sb = cpool.tile([128, 2], F32)
            ones_sb = cpool.tile([K, 1], F32)
            Sel_sb = cpool.tile([128, 2], BF16)
            aconst = cpool.tile([128, 1], F32)
            nc.vector.memset(aconst[:], TANH_A)

            gp = nc.gpsimd if hasattr(nc.gpsimd, "tensor_copy") else nc.vector

            def tanh_dve(dst, src):
                nc.vector._custom_dve(
                    TANH_PC, out=dst, in0=src, in1=aconst[:],
                    s0=TANH_B, s1=TANH_CB, imm2=TANH_CC,
                )

            def tanh_act(dst, src):
                nc.scalar.activation(dst, src, AFT.Tanh)

            NP = TILES * NPAIR          # 128 pairs, software-pipelined flat
            h1s_of, h2s_of, lg_of, ex_of = {}, {}, {}, {}
            tile_io = {}                # t -> (xt_bf, E_sb, SD_S, SD_D)
            pend_epi = [None]           # (E_sb, SD_S, SD_D, col)
            pend_acc = [None]           # (tt2, col)

            def prologue(t):
                """Input DMAs + cast + q matmuls + E exp for tile t."""
                col = slice(t * NT, (t + 1) * NT)
                xt = iop.tile([P + 1, NT], F32, tag="xt")
                nc.sync.dma_start(xt[:], xT[:, col])
                xs = iop.tile([P, NT], F32, tag="xs")
                nc.sync.dma_start(xs[:], xsqT[:, col])
                xt_bf = iop.tile([P + 1, NT], BF16, tag="xtb")
                gp.tensor_copy(xt_bf[:], xt[:])

                if t == 0:
                    def a_chunk(ch):
                        nc.sync.dma_start(
                            A_sb[:, ch * ACH : (ch + 1) * ACH],
                            A_all[:, ch * ACH : (ch + 1) * ACH],
                        )

                    WCH = NPAIR * 256 // 4
                    a_chunk(0)
                    nc.sync.dma_start(W2_sb[:], W2DR[:])
                    nc.sync.dma_start(W3_sb[:, :, 0:WCH], W3DR[:, :, 0:WCH])
                    nc.sync.dma_start(B3r_sb[:], B3R[:])
                    a_chunk(1)
                    nc.sync.dma_start(Sel_sb[:], Sel[:])
                    for ch in range(2, 8):
                        a_chunk(ch)
                        if ch <= 4:
                            w0 = (ch - 1) * WCH
                            nc.sync.dma_start(
                                W3_sb[:, :, w0 : w0 + WCH],
                                W3DR[:, :, w0 : w0 + WCH],
                            )
                    nc.sync.dma_start(B2_sb[:], B2h[:])
                    nc.sync.dma_start(ones_sb[:], ones[:])

                # E = exp(-0.5*(U.x^2 + V.x) + B_k); q borrows an lg-ring bank
                q_ps = plg.tile([128, NT], F32, tag="lg")
                nc.tensor.matmul(
                    q_ps[0:K, :], UV_sb[:, 0:K], xs[:], start=True, stop=False
                )
                nc.tensor.matmul(
                    q_ps[0:K, :], UV_sb[:, K : 2 * K], xt[0:P, :],
                    start=False, stop=True,
                )
                E_sb = edp.tile([K, NT], F32, tag="E")
                nc.scalar.activation(
                    E_sb[:], q_ps[0:K, :], AFT.Exp, bias=BEx_sb[:], scale=-0.5
                )
                SD_S = edp.tile([K, NT], F32, tag="SDs")
                SD_D = edp.tile([K, NT], BF16, tag="SDd")
                tile_io[t] = (xt_bf, E_sb, SD_S, SD_D)
                return col

            col_of = {0: prologue(0)}

            def stage_mm1_tanh1(p):
                """mm1 (bf16, augmented bias row) + tanh1 for both comps."""
                xt_bf = tile_io[p // NPAIR][0]
                for par in range(2):
                    k = 2 * (p % NPAIR) + par
                    h1p = pmlp.tile([128, 2, NT], F32, tag="mlp")
                    for half in range(2):
                        nc.tensor.matmul(
                            h1p[:, half, :],
                            A_sb[:, k * H1 + half * 128 : k * H1 + (half + 1) * 128],
                            xt_bf[:],
                            start=True,
                            stop=True,
                        )
                    h1s = h1pool.tile([128, 2, NT], F8, tag="h1s")
                    if par == 0:
                        tanh_act(h1s[:, :, :], h1p[:, :, :])
                    else:
                        tanh_dve(h1s[:, :, :], h1p[:, :, :])
                    h1s_of[(p, par)] = h1s

            def stage_mm2_tanh2(p):
                """fp8 DoubleRow mm2 + tanh2 for both comps."""
                for par in range(2):
                    h1s = h1s_of.pop((p, par))
                    h2p = pmlp.tile([128, 2, NT], F32, tag="mlp")
                    for v in range(2):
                        nc.tensor.matmul(
                            h2p[:, v, :],
                            W2_sb[:, :, v * 128 : (v + 1) * 128],
                            h1s[:, :, :],
                            start=True,
                            stop=True,
                            perf_mode=DR,
                        )
                    h2s = h2pool.tile([128, 2, NT], F8, tag="h2s")
                    if use_b2:
                        for v in range(2):
                            nc.scalar.activation(
                                h2s[:, v, :], h2p[:, v, :], AFT.Tanh,
                                bias=B2_sb[:, v : v + 1],
                            )
                    elif par == 0:
                        tanh_dve(h2s[:, :, :], h2p[:, :, :])
                    else:
                        tanh_act(h2s[:, :, :], h2p[:, :, :])
                    h2s_of[(p, par)] = h2s

            def stage_mm3_exp(p):
                """fp8 DR logits + exp.  W3 is rotated per component so the
                softmax diagonal lands at partition 0 (even) / 64 (odd), and
                padded per parity so the pair accumulates into one full-height
                PSUM bank (DR requires out base partition 0)."""
                pg = p % NPAIR
                lg2 = plg.tile([128, NT], F32, tag="lg")
                nc.tensor.matmul(
                    lg2[:], W3_sb[:, :, pg * 256 : pg * 256 + 128],
                    h2s_of.pop((p, 0))[:, :, :],
                    start=True, stop=False, perf_mode=DR,
                )
                nc.tensor.matmul(
                    lg2[:], W3_sb[:, :, pg * 256 + 128 : pg * 256 + 256],
                    h2s_of.pop((p, 1))[:, :, :],
                    start=False, stop=True, perf_mode=DR,
                )
                ex2 = explp.tile([128, NT], BF16, tag="ex")
                nc.scalar.activation(
                    ex2[:], lg2[:], AFT.Exp, bias=B3r_sb[:, pg : pg + 1]
                )
                sdd = tile_io[p // NPAIR][3]
                nc.sync.dma_start(sdd[2 * pg : 2 * pg + 1, :], ex2[0:1, :])
                nc.sync.dma_start(sdd[2 * pg + 1 : 2 * pg + 2, :], ex2[64:65, :])
                ex_of[p] = ex2

            coll_of = [None]     # current collector psum tile

            def stage_sel(p):
                """S selector matmul into a 3-pair PSUM collector (out base
                partition limited to 0/32/64); on the last block, reciprocal
                is applied directly to the collector and the 1/S rows are
                scattered into sinv64 by DMA."""
                ex2 = ex_of.pop(p)
                pg = p % NPAIR
                c, b = pg // 3, pg % 3
                if b == 0:
                    coll_new = pcoll.tile([128, NT], F32, tag="coll")
                    coll_of[0] = coll_new
                coll = coll_of[0]
                nc.tensor.matmul(
                    coll[32 * b : 32 * b + 2, :], Sel_sb[:], ex2[:],
                    start=True, stop=True,
                )
                last = pg == NPAIR - 1
                if b == 2 or last:
                    rinv = rvp.tile([128, NT], F32, tag="rv")
                    hi = 32 * b + 2
                    nc.vector.reciprocal_approx_fast(rinv[0:hi, :], coll[0:hi, :])
                    sinv = tile_io[p // NPAIR][2]
                    for bb in range(b + 1):
                        nc.sync.dma_start(
                            sinv[6 * c + 2 * bb : 6 * c + 2 * bb + 2, :],
                            rinv[32 * bb : 32 * bb + 2, :],
                        )

            def flush_epi_dve():
                if pend_epi[0] is None:
                    return
                E_prev, sinv, sdd, pcol = pend_epi[0]
                pend_epi[0] = None
                tt = tmpp.tile([K, NT], F32, tag="tmp2")
                gp.tensor_mul(tt[:], sdd[:], sinv[:])
                tt2 = tmpp.tile([K, NT], F32, tag="tt2")
                gp.tensor_mul(tt2[:], tt[:], E_prev[:])
                pend_acc[0] = (tt2, pend_col[0])

            def flush_acc():
                if pend_acc[0] is None:
                    return
                tt2, acc_col = pend_acc[0]
                pend_acc[0] = None
                acc_ps = plg.tile([128, NT], F32, tag="lg")
                nc.tensor.matmul(
                    acc_ps[0:1, :], ones_sb[:], tt2[:], start=True, stop=True
                )
                acc_sb = tmpp.tile([1, NT], F32, tag="acc")
                nc.vector.tensor_copy(acc_sb[:], acc_ps[0:1, :])
                nc.sync.dma_start(acc_out[0:1, acc_col], acc_sb[:])

            pend_col = [None]

            for it in range(NP + 3):
                if it >= 3:
                    stage_sel(it - 3)
                if it % NPAIR == 16:
                    flush_acc()
                if 1 <= it <= NP:
                    stage_mm2_tanh2(it - 1)
                if it < NP:
                    stage_mm1_tanh1(it)
                if 2 <= it <= NP + 1:
                    stage_mm3_exp(it - 2)
                if it < NP and it % NPAIR == NPAIR - 4:
                    t = it // NPAIR
                    if t + 1 < TILES:
                        col_of[t + 1] = prologue(t + 1)
                if it % NPAIR == NPAIR - 1:
                    t = it // NPAIR
                    _, E_sb, SD_S, SD_D = tile_io[t]
                    pend_epi[0] = (E_sb, SD_S, SD_D, col_of[t])
                if it % NPAIR == 8 and it > NPAIR:
                    pend_col[0] = col_of[it // NPAIR - 1]
                    flush_epi_dve()

            # tail: epilogue for the last tile
            pend_col[0] = col_of[TILES - 1]
            flush_epi_dve()
            flush_acc()

    nc.finalize()
    return nc


def _prep_consts(m, log_s, W1, b1, W2, b2, W3, b3):
    import ml_dtypes

    bf16 = ml_dtypes.bfloat16
    f8 = ml_dtypes.float8_e4m3fn
    inv_s = np.exp(-np.asarray(log_s, np.float64))          # [K,P]
    m64 = np.asarray(m, np.float64)
    W1_64 = np.asarray(W1, np.float64)
    ims = inv_s * m64                                       # [K,P]

    # A_all[p, k*H1+h] = W1[h,p]*inv_s[k,p]; row P = c1_k[h]
    A = W1_64[None, :, :] * inv_s[:, None, :]               # [K,H1,P]
    A_all = np.empty((P + 1, K * H1), np.float32)
    A_all[:P] = A.transpose(2, 0, 1).reshape(P, K * H1)
    c1 = np.asarray(b1, np.float64)[None, :] - np.einsum("hp,kp->kh", W1_64, ims)
    A_all[P] = c1.reshape(K * H1).astype(np.float32)

    # DoubleRow fp8 weights: lhsT[p, j, o] = W[o, p + 128j]
    W2a = np.asarray(W2, np.float32)
    W2dr = np.empty((128, 2, 256), np.float32)
    for j in range(2):
        for v in range(2):
            W2dr[:, j, v * 128 : (v + 1) * 128] = W2a[
                v * 128 : (v + 1) * 128, 128 * j : 128 * (j + 1)
            ].T
    # W3 per-pair stationaries: rotated per component so comp k's diagonal
    # class lands at out partition 0 (even) / 64 (odd), padded to 128 out
    # columns per parity (DR requires out base partition 0; the pair
    # accumulates into one full-height PSUM bank).
    W3a = np.asarray(W3, np.float32)
    W3dr = np.zeros((128, 2, NPAIR * 256), np.float32)
    cidx = np.arange(64)
    for pg in range(NPAIR):
        for par in range(2):
            k = 2 * pg + par
            rot = W3a[(cidx + k) % 64, :]          # [c', 256]
            for j in range(2):
                base = pg * 256 + 192 * par
                W3dr[:, j, base : base + 64] = rot[:, 128 * j : 128 * (j + 1)].T

    UV = np.empty((P, 2 * K), np.float32)
    UV[:, 0:K] = (inv_s**2).T
    UV[:, K : 2 * K] = (-2.0 * m64 * inv_s**2).T

    w_k = np.sum(ims**2, axis=1)                            # [K]
    log_det = -np.asarray(log_s, np.float64).sum(axis=1)    # [K]
    BEx = (-0.5 * w_k - 0.5 * P * LOG2PI + log_det + C_OFF).astype(np.float32)

    b3a = np.asarray(b3, np.float32)
    B3R = np.empty((128, NPAIR), np.float32)
    for pg in range(NPAIR):
        B3R[0:64, pg] = b3a[(cidx + 2 * pg) % 64]
        B3R[64:128, pg] = b3a[(cidx + 2 * pg + 1) % 64]
    B2h = np.stack([np.asarray(b2)[:128], np.asarray(b2)[128:]], axis=1).astype(
        np.float32
    )

    Sel = np.zeros((128, 2), np.float32)
    Sel[0:64, 0] = 1.0
    Sel[64:128, 1] = 1.0

    return {
        "A_all": A_all.astype(bf16),
        "W2DR": W2dr.astype(f8),
        "W3DR": W3dr.astype(f8),
        "UV": UV,
        "BEx": BEx.reshape(K, 1),
        "B3R": B3R,
        "B2h": B2h,
        "Sel": Sel.astype(bf16),
        "ones": np.ones((K, 1), np.float32),
    }


def kernel(x, m, log_s, W1, b1, W2, b2, W3, b3):
    x = np.asarray(x, np.float32)
    consts = _prep_consts(m, log_s, W1, b1, W2, b2, W3, b3)
    use_b2 = bool(np.any(np.asarray(b2)))

    key = ("prog", use_b2)
    if key not in _cached:
        _cached[key] = _build_program(use_b2)
    nc = _cached[key]

    xT = np.empty((P + 1, N), np.float32)
    xT[:P] = x.T
    xT[P] = 1.0
    xsqT = (x.T.astype(np.float64) ** 2).astype(np.float32)

    in_maps = []
    for i in range(NCORES):
        col = slice(i * RPC, (i + 1) * RPC)
        im = {"xT": np.ascontiguousarray(xT[:, col]),
              "xsqT": np.ascontiguousarray(xsqT[:, col])}
        im.update(consts)
        in_maps.append(im)

    res = bass_utils.run_bass_kernel_spmd(
        nc, in_maps, list(range(NCORES)), trace=TRACE
    )
    global LAST_RESULT
    LAST_RESULT = res
    acc = np.concatenate([r["acc_out"].reshape(RPC) for r in res.results])
    return (np.log(acc.astype(np.float64)) - C_OFF).astype(np.float32)


# revision 19
# speedup vs baseline: 1.5378x; 1.0197x over previous
"""Trainium2 Bass kernel for nn_DIFLayer (deep invertible flow layer).

Math (per row n of x, K=64 mixture components, P=64 dims, H1=H2=256):
    z_k = (x - m_k) * exp(-log_s_k)
    ref_lp_k = -0.5*||z_k||^2 - 0.5*P*log(2pi)
    h1 = tanh(W1 z_k + b1); h2 = tanh(W2 h1 + b2); logits = W3 h2 + b3
    lv_k = ref_lp_k + log_softmax(logits)[k] + logdet_k
    out = logsumexp_k(lv_k)

Device restructure:
    mm1 folds the flow into per-component weights A_k = W1*diag(inv_s_k)
    with an augmented bias contraction row (bf16).  tanh1/tanh2 are split
    between the ACT engine (exact) and a custom single-instruction DVE op
    (clamped odd deg-5 polynomial, max abs err 1.4e-2), both writing fp8e4
    directly.  mm2/mm3 run in fp8e4 with DoubleRow perf mode (2 contraction
    rows per PE cell).  ref_lp via q = U.x^2 + V.x (fp32 matmuls),
    E = exp(-0.5q + B_k).  Softmax handled unnormalized: expl = exp(logits
    + b3); per-pair selector matmuls extract S_k = sum_c expl and the diag
    D_k into a shared PSUM collector (4 pairs per bank), batch-copied to
    SBUF and row-scattered by DMA.  out = log(sum_k E*D/S) - C  (log on
    host).

Sharded data-parallel over rows: 8 cores x 2048 rows.
"""

import numpy as np

import concourse.bacc as bacc
import concourse.bass as bass
import concourse.mybir as mybir
import concourse.tile as tile
from concourse import bass_utils

# --------------------------------------------------------------------------
# Custom DVE op: tanh(u) ~= uc*(a + v*(b + v*c)), v = uc^2, uc = clamp(u,+-B)
# One DVE instruction per tile (8 ALU stages), runs concurrently with the
# ACT engine's exact tanh.  Registered at import time.
# --------------------------------------------------------------------------
from concourse.dve_spec import (
    Spec, Src0, C0, C1, C2, C3, Zero, maxx, minn, sq, _spill_c3_to_src1,
)
import concourse.dve_ops as _dve_ops_mod
from concourse.dve_ops import DveOp

TANH_B = 2.1350599
TANH_A = 0.94666379
TANH_CB = -0.19501118
TANH_CC = 0.01945195


def _tanh_pc_ref(in0, in1, s0, s1, imm2):
    prt = in0.shape[0]
    ucv = np.clip(
        in0.astype(np.float32),
        -np.asarray(s0, np.float32),
        np.asarray(s0, np.float32),
    )
    vv = ucv * ucv
    a = np.asarray(in1, np.float32).reshape(prt, -1)[:, :1]
    return ((vv * imm2 + s1) * vv + a) * ucv


_uc = maxx(minn(Src0, C0), Zero - C0)
_v = sq(_uc)
_body = _spill_c3_to_src1(((_v * C2 + C1) * _v + C3) * _uc)
TANH_PC = DveOp(
    "TANH_PC",
    Spec(body=_body, reference=_tanh_pc_ref),
    subdim=False,
    uops_sha={"v3": "b46f8204b307e3bf", "v4": "e95adf23d01b2e24"},
)
if TANH_PC.name not in _dve_ops_mod._SUB_OPCODE_FOR_NAME:
    _dve_ops_mod.OPS.append(TANH_PC)
    _dve_ops_mod._SUB_OPCODE_FOR_NAME[TANH_PC.name] = (
        _dve_ops_mod._CUSTOM_DVE_ROW_BASE + len(_dve_ops_mod.OPS) - 1
    )
    _dve_ops_mod.CUSTOM_DVE_SPECS[TANH_PC.name] = TANH_PC.spec

F32 = mybir.dt.float32
BF16 = mybir.dt.bfloat16
F8 = mybir.dt.float8e4
AFT = mybir.ActivationFunctionType
DR = mybir.MatmulPerfMode.DoubleRow

N, K, P = 16384, 64, 64
H1, H2 = 256, 256
NCORES = 8
RPC = N // NCORES          # rows per core = 2048
NT = 512                   # rows per n-tile (free dim)
TILES = RPC // NT          # 4
NPAIR = K // 2             # 32 component pairs
NGRP = NPAIR // 2          # 16 groups of 2 pairs (4 components)
LOG2PI = float(np.log(2.0 * np.pi))
C_OFF = 115.0              # global exp offset; keeps exp(lv + C) in fp32 range

_cached = {}
TRACE = False          # set by test harness to capture an NTFF profile
LAST_RESULT = None     # BassKernelResults of the most recent run


def _build_program(use_b2: bool):
    nc = bacc.Bacc("TRN2", target_bir_lowering=False, debug=False)

    xT = nc.dram_tensor("xT", [P + 1, RPC], F32, kind="ExternalInput")
    xsqT = nc.dram_tensor("xsqT", [P, RPC], F32, kind="ExternalInput")
    A_all = nc.dram_tensor("A_all", [P + 1, K * H1], BF16, kind="ExternalInput")
    W2DR = nc.dram_tensor("W2DR", [128, 2, 256], F8, kind="ExternalInput")
    W3DR = nc.dram_tensor("W3DR", [128, 2, NPAIR * 256], F8, kind="ExternalInput")
    UV = nc.dram_tensor("UV", [P, 2 * K], F32, kind="ExternalInput")
    BEx = nc.dram_tensor("BEx", [K, 1], F32, kind="ExternalInput")
    B3R = nc.dram_tensor("B3R", [128, NPAIR], F32, kind="ExternalInput")
    B2h = nc.dram_tensor("B2h", [128, 2], F32, kind="ExternalInput")
    Sel = nc.dram_tensor("Sel", [128, 2], BF16, kind="ExternalInput")
    ones = nc.dram_tensor("ones", [K, 1], F32, kind="ExternalInput")
    acc_out = nc.dram_tensor("acc_out", [1, RPC], F32, kind="ExternalOutput")

    with tile.TileContext(nc) as tc:
        with (
            tc.tile_pool(name="const", bufs=1) as cpool,
            tc.tile_pool(name="io", bufs=3) as iop,
            tc.tile_pool(name="h1pool", bufs=4) as h1pool,
            tc.tile_pool(name="h2pool", bufs=4) as h2pool,
            tc.tile_pool(name="expl", bufs=4) as explp,
            tc.tile_pool(name="ed", bufs=2) as edp,
            tc.tile_pool(name="rv", bufs=3) as rvp,
            tc.tile_pool(name="tmp", bufs=2) as tmpp,
            tc.tile_pool(name="pmlp", bufs=2, space="PSUM") as pmlp,
            tc.tile_pool(name="plg", bufs=2, space="PSUM") as plg,
            tc.tile_pool(name="pcoll", bufs=2, space="PSUM") as pcoll,
        ):
            # --- constants; small hot ones first so tile 0 can start ---
            UV_sb = cpool.tile([P, 2 * K], F32)
            nc.sync.dma_start(UV_sb[:], UV[:])
            BEx_sb = cpool.tile([K, 1], F32)
            nc.sync.dma_start(BEx_sb[:], BEx[:])
            A_sb = cpool.tile([P + 1, K * H1], BF16)
            ACH = K * H1 // 8
            W2_sb = cpool.tile([128, 2, 256], F8)
            W3_sb = cpool.tile([128, 2, NPAIR * 256], F8)
            B3r_sb = cpool.tile([128, NPAIR], F32)
            B2_sb = cpool.tile([128, 2], F32)
            ones_sb = cpool.tile([K, 1], F32)
            Sel_sb = cpool.tile([128, 2], BF16)
            aconst = cpool.tile([128, 1], F32)
            nc.vector.memset(aconst[:], TANH_A)

            gp = nc.gpsimd if hasattr(nc.gpsimd, "tensor_copy") else nc.vector

            def tanh_dve(dst, src):
                nc.vector._custom_dve(
                    TANH_PC, out=dst, in0=src, in1=aconst[:],
                    s0=TANH_B, s1=TANH_CB, imm2=TANH_CC,
                )

            def tanh_act(dst, src):
                nc.scalar.activation(dst, src, AFT.Tanh)

            NP = TILES * NPAIR          # 128 pairs, software-pipelined flat
            h1s_of, h2s_of, lg_of, ex_of = {}, {}, {}, {}
            tile_io = {}                # t -> (xt_bf, E_sb, SD_S, SD_D)
            pend_epi = [None]           # (E_sb, SD_S, SD_D, col)
            pend_acc = [None]           # (tt2, col)

            def prologue(t):
                """Input DMAs + cast + q matmuls + E exp for tile t."""
                col = slice(t * NT, (t + 1) * NT)
                xt = iop.tile([P + 1, NT], F32, tag="xt")
                nc.sync.dma_start(xt[:], xT[:, col])
                xs = iop.tile([P, NT], F32, tag="xs")
                nc.sync.dma_start(xs[:], xsqT[:, col])
                xt_bf = iop.tile([P + 1, NT], BF16, tag="xtb")
                gp.tensor_copy(xt_bf[:], xt[:])

                if t == 0:
                    def a_chunk(ch):
                        nc.sync.dma_start(
                            A_sb[:, ch * ACH : (ch + 1) * ACH],
                            A_all[:, ch * ACH : (ch + 1) * ACH],
                        )

                    WCH = NPAIR * 256 // 4
                    a_chunk(0)
                    nc.sync.dma_start(W2_sb[:], W2DR[:])
                    nc.sync.dma_start(W3_sb[:, :, 0:WCH], W3DR[:, :, 0:WCH])
                    nc.sync.dma_start(B3r_sb[:], B3R[:])
                    a_chunk(1)
                    nc.sync.dma_start(Sel_sb[:], Sel[:])
                    for ch in range(2, 8):
                        a_chunk(ch)
                        if ch <= 4:
                            w0 = (ch - 1) * WCH
                            nc.sync.dma_start(
                                W3_sb[:, :, w0 : w0 + WCH],
                                W3DR[:, :, w0 : w0 + WCH],
                            )
                    nc.sync.dma_start(B2_sb[:], B2h[:])
                    nc.sync.dma_start(ones_sb[:], ones[:])

                # E = exp(-0.5*(U.x^2 + V.x) + B_k); q borrows an lg-ring bank
                q_ps = plg.tile([128, NT], F32, tag="lg")
                nc.tensor.matmul(
                    q_ps[0:K, :], UV_sb[:, 0:K], xs[:], start=True, stop=False
                )
                nc.tensor.matmul(
                    q_ps[0:K, :], UV_sb[:, K : 2 * K], xt[0:P, :],
                    start=False, stop=True,
                )
                E_sb = edp.tile([K, NT], F32, tag="E")
                nc.scalar.activation(
                    E_sb[:], q_ps[0:K, :], AFT.Exp, bias=BEx_sb[:], scale=-0.5
                )
                SD_S = edp.tile([K, NT], F32, tag="SDs")
                SD_D = edp.tile([K, NT], BF16, tag="SDd")
                tile_io[t] = (xt_bf, E_sb, SD_S, SD_D)
                return col

            col_of = {0: prologue(0)}

            def stage_mm1_tanh1(p):
                """mm1 (bf16, augmented bias row) + tanh1 for both comps."""
                xt_bf = tile_io[p // NPAIR][0]
                for par in range(2):
                    k = 2 * (p % NPAIR) + par
                    h1p = pmlp.tile([128, 2, NT], F32, tag="mlp")
                    for half in range(2):
                        nc.tensor.matmul(
                            h1p[:, half, :],
                            A_sb[:, k * H1 + half * 128 : k * H1 + (half + 1) * 128],
                            xt_bf[:],
                            start=True,
                            stop=True,
                        )
                    h1s = h1pool.tile([128, 2, NT], F8, tag="h1s")
                    if par == 0:
                        tanh_act(h1s[:, :, :], h1p[:, :, :])
                    else:
                        tanh_dve(h1s[:, :, :], h1p[:, :, :])
                    h1s_of[(p, par)] = h1s

            def stage_mm2_tanh2(p):
                """fp8 DoubleRow mm2 + tanh2 for both comps."""
                for par in range(2):
                    h1s = h1s_of.pop((p, par))
                    h2p = pmlp.tile([128, 2, NT], F32, tag="mlp")
                    for v in range(2):
                        nc.tensor.matmul(
                            h2p[:, v, :],
                            W2_sb[:, :, v * 128 : (v + 1) * 128],
                            h1s[:, :, :],
                            start=True,
                            stop=True,
                            perf_mode=DR,
                        )
                    h2s = h2pool.tile([128, 2, NT], F8, tag="h2s")
                    if use_b2:
                        for v in range(2):
                            nc.scalar.activation(
                                h2s[:, v, :], h2p[:, v, :], AFT.Tanh,
                                bias=B2_sb[:, v : v + 1],
                            )
                    elif par == 0:
                        tanh_dve(h2s[:, :, :], h2p[:, :, :])
                    else:
                        tanh_act(h2s[:, :, :], h2p[:, :, :])
                    h2s_of[(p, par)] = h2s

            def stage_mm3_exp(p):
                """fp8 DR logits + exp.  W3 is rotated per component so the
                softmax diagonal lands at partition 0 (even) / 64 (odd), and
                padded per parity so the pair accumulates into one full-height
                PSUM bank (DR requires out base partition 0)."""
                pg = p % NPAIR
                lg2 = plg.tile([128, NT], F32, tag="lg")
                nc.tensor.matmul(
                    lg2[:], W3_sb[:, :, pg * 256 : pg * 256 + 128],
                    h2s_of.pop((p, 0))[:, :, :],
                    start=True, stop=False, perf_mode=DR,
                )
                nc.tensor.matmul(
                    lg2[:], W3_sb[:, :, pg * 256 + 128 : pg * 256 + 256],
                    h2s_of.pop((p, 1))[:, :, :],
                    start=False, stop=True, perf_mode=DR,
                )
                ex2 = explp.tile([128, NT], BF16, tag="ex")
                nc.scalar.activation(
                    ex2[:], lg2[:], AFT.Exp, bias=B3r_sb[:, pg : pg + 1]
                )
                sdd = tile_io[p // NPAIR][3]
                nc.sync.dma_start(sdd[2 * pg : 2 * pg + 1, :], ex2[0:1, :])
                nc.sync.dma_start(sdd[2 * pg + 1 : 2 * pg + 2, :], ex2[64:65, :])
                ex_of[p] = ex2

            coll_of = [None]     # current collector psum tile

            def stage_sel(p):
                """S selector matmul into a 3-pair PSUM collector (out base
                partition limited to 0/32/64); on the last block, reciprocal
                is applied directly to the collector and the 1/S rows are
                scattered into sinv64 by DMA."""
                ex2 = ex_of.pop(p)
                pg = p % NPAIR
                c, b = pg // 3, pg % 3
                if b == 0:
                    coll_new = pcoll.tile([128, NT], F32, tag="coll")
                    coll_of[0] = coll_new
                coll = coll_of[0]
                nc.tensor.matmul(
                    coll[32 * b : 32 * b + 2, :], Sel_sb[:], ex2[:],
                    start=True, stop=True,
                )
                last = pg == NPAIR - 1
                if b == 2 or last:
                    rinv = rvp.tile([128, NT], F32, tag="rv")
                    hi = 32 * b + 2
                    nc.vector.reciprocal_approx_fast(rinv[0:hi, :], coll[0:hi, :])
                    sinv = tile_io[p // NPAIR][2]
                    for bb in range(b + 1):
                        nc.sync.dma_start(
                            sinv[6 * c + 2 * bb : 6 * c + 2 * bb + 2, :],
                            rinv[32 * bb : 32 * bb + 2, :],
                        )

            def flush_epi_dve():
                if pend_epi[0] is None:
                    return
                E_prev, sinv, sdd, pcol = pend_epi[0]
                pend_epi[0] = None
                tt = tmpp.tile([K, NT], F32, tag="tmp2")
                gp.tensor_mul(tt[:], sdd[:], sinv[:])
                tt2 = tmpp.tile([K, NT], F32, tag="tt2")
                gp.tensor_mul(tt2[:], tt[:], E_prev[:])
                pend_acc[0] = (tt2, pend_col[0])

            def flush_acc():
                if pend_acc[0] is None:
                    return
                tt2, acc_col = pend_acc[0]
                pend_acc[0] = None
                acc_ps = plg.tile([128, NT], F32, tag="lg")
                nc.tensor.matmul(
                    acc_ps[0:1, :], ones_sb[:], tt2[:], start=True, stop=True
                )
                acc_sb = tmpp.tile([1, NT], F32, tag="acc")
                nc.vector.tensor_copy(acc_sb[:], acc_ps[0:1, :])
                nc.sync.dma_start(acc_out[0:1, acc_col], acc_sb[:])

            pend_col = [None]

            for it in range(NP + 3):
                if 1 <= it <= NP:
                    stage_mm2_tanh2(it - 1)
                if it >= 3:
                    stage_sel(it - 3)
                if it % NPAIR == 16:
                    flush_acc()
                if it < NP:
                    stage_mm1_tanh1(it)
                if 2 <= it <= NP + 1:
                    stage_mm3_exp(it - 2)
                if it < NP and it % NPAIR == NPAIR - 8:
                    t = it // NPAIR
                    if t + 1 < TILES:
                        col_of[t + 1] = prologue(t + 1)
                if it % NPAIR == NPAIR - 1:
                    t = it // NPAIR
                    _, E_sb, SD_S, SD_D = tile_io[t]
                    pend_epi[0] = (E_sb, SD_S, SD_D, col_of[t])
                if it % NPAIR == 8 and it > NPAIR:
                    pend_col[0] = col_of[it // NPAIR - 1]
                    flush_epi_dve()

            # tail: epilogue for the last tile
            pend_col[0] = col_of[TILES - 1]
            flush_epi_dve()
            flush_acc()

    nc.finalize()
    return nc


def _prep_consts(m, log_s, W1, b1, W2, b2, W3, b3):
    import ml_dtypes

    bf16 = ml_dtypes.bfloat16
    f8 = ml_dtypes.float8_e4m3fn
    inv_s = np.exp(-np.asarray(log_s, np.float64))          # [K,P]
    m64 = np.asarray(m, np.float64)
    W1_64 = np.asarray(W1, np.float64)
    ims = inv_s * m64                                       # [K,P]

    # A_all[p, k*H1+h] = W1[h,p]*inv_s[k,p]; row P = c1_k[h]
    A = W1_64[None, :, :] * inv_s[:, None, :]               # [K,H1,P]
    A_all = np.empty((P + 1, K * H1), np.float32)
    A_all[:P] = A.transpose(2, 0, 1).reshape(P, K * H1)
    c1 = np.asarray(b1, np.float64)[None, :] - np.einsum("hp,kp->kh", W1_64, ims)
    A_all[P] = c1.reshape(K * H1).astype(np.float32)

    # DoubleRow fp8 weights: lhsT[p, j, o] = W[o, p + 128j]
    W2a = np.asarray(W2, np.float32)
    W2dr = np.empty((128, 2, 256), np.float32)
    for j in range(2):
        for v in range(2):
            W2dr[:, j, v * 128 : (v + 1) * 128] = W2a[
                v * 128 : (v + 1) * 128, 128 * j : 128 * (j + 1)
            ].T
    # W3 per-pair stationaries: rotated per component so comp k's diagonal
    # class lands at out partition 0 (even) / 64 (odd), padded to 128 out
    # columns per parity (DR requires out base partition 0; the pair
    # accumulates into one full-height PSUM bank).
    W3a = np.asarray(W3, np.float32)
    W3dr = np.zeros((128, 2, NPAIR * 256), np.float32)
    cidx = np.arange(64)
    for pg in range(NPAIR):
        for par in range(2):
            k = 2 * pg + par
            rot = W3a[(cidx + k) % 64, :]          # [c', 256]
            for j in range(2):
                base = pg * 256 + 192 * par
                W3dr[:, j, base : base + 64] = rot[:, 128 * j : 128 * (j + 1)].T

    UV = np.empty((P, 2 * K), np.float32)
    UV[:, 0:K] = (inv_s**2).T
    UV[:, K : 2 * K] = (-2.0 * m64 * inv_s**2).T

    w_k = np.sum(ims**2, axis=1)                            # [K]
    log_det = -np.asarray(log_s, np.float64).sum(axis=1)    # [K]
    BEx = (-0.5 * w_k - 0.5 * P * LOG2PI + log_det + C_OFF).astype(np.float32)

    b3a = np.asarray(b3, np.float32)
    B3R = np.empty((128, NPAIR), np.float32)
    for pg in range(NPAIR):
        B3R[0:64, pg] = b3a[(cidx + 2 * pg) % 64]
        B3R[64:128, pg] = b3a[(cidx + 2 * pg + 1) % 64]
    B2h = np.stack([np.asarray(b2)[:128], np.asarray(b2)[128:]], axis=1).astype(
        np.float32
    )

    Sel = np.zeros((128, 2), np.float32)
    Sel[0:64, 0] = 1.0
    Sel[64:128, 1] = 1.0

    return {
        "A_all": A_all.astype(bf16),
        "W2DR": W2dr.astype(f8),
        "W3DR": W3dr.astype(f8),
        "UV": UV,
        "BEx": BEx.reshape(K, 1),
        "B3R": B3R,
        "B2h": B2h,
        "Sel": Sel.astype(bf16),
        "ones": np.ones((K, 1), np.float32),
    }


def kernel(x, m, log_s, W1, b1, W2, b2, W3, b3):
    x = np.asarray(x, np.float32)
    consts = _prep_consts(m, log_s, W1, b1, W2, b2, W3, b3)
    use_b2 = bool(np.any(np.asarray(b2)))

    key = ("prog", use_b2)
    if key not in _cached:
        _cached[key] = _build_program(use_b2)
    nc = _cached[key]

    xT = np.empty((P + 1, N), np.float32)
    xT[:P] = x.T
    xT[P] = 1.0
    xsqT = (x.T.astype(np.float64) ** 2).astype(np.float32)

    in_maps = []
    for i in range(NCORES):
        col = slice(i * RPC, (i + 1) * RPC)
        im = {"xT": np.ascontiguousarray(xT[:, col]),
              "xsqT": np.ascontiguousarray(xsqT[:, col])}
        im.update(consts)
        in_maps.append(im)

    res = bass_utils.run_bass_kernel_spmd(
        nc, in_maps, list(range(NCORES)), trace=TRACE
    )
    global LAST_RESULT
    LAST_RESULT = res
    acc = np.concatenate([r["acc_out"].reshape(RPC) for r in res.results])
    return (np.log(acc.astype(np.float64)) - C_OFF).astype(np.float32)


# revision 20
# speedup vs baseline: 1.7012x; 1.1062x over previous
"""Trainium2 Bass kernel for nn_DIFLayer (deep invertible flow layer).

Math (per row n of x, K=64 mixture components, P=64 dims, H1=H2=256):
    z_k = (x - m_k) * exp(-log_s_k)
    ref_lp_k = -0.5*||z_k||^2 - 0.5*P*log(2pi)
    h1 = tanh(W1 z_k + b1); h2 = tanh(W2 h1 + b2); logits = W3 h2 + b3
    lv_k = ref_lp_k + log_softmax(logits)[k] + logdet_k
    out = logsumexp_k(lv_k)

Device restructure:
    mm1 folds the flow into per-component weights A_k = W1*diag(inv_s_k)
    with an augmented bias contraction row (bf16).  tanh1/tanh2 are split
    between the ACT engine (exact) and a custom single-instruction DVE op
    (clamped odd deg-5 polynomial, max abs err 1.4e-2), both writing fp8e4
    directly.  mm2/mm3 run in fp8e4 with DoubleRow perf mode (2 contraction
    rows per PE cell).  ref_lp via q = U.x^2 + V.x (fp32 matmuls),
    E = exp(-0.5q + B_k).  Softmax handled unnormalized: expl = exp(logits
    + b3); per-pair selector matmuls extract S_k = sum_c expl and the diag
    D_k into a shared PSUM collector (4 pairs per bank), batch-copied to
    SBUF and row-scattered by DMA.  out = log(sum_k E*D/S) - C  (log on
    host).

Sharded data-parallel over rows: 8 cores x 2048 rows.
"""

import numpy as np

import concourse.bacc as bacc
import concourse.bass as bass
import concourse.mybir as mybir
import concourse.tile as tile
from concourse import bass_utils

# --------------------------------------------------------------------------
# Custom DVE op: tanh(u) ~= uc*(a + v*(b + v*c)), v = uc^2, uc = clamp(u,+-B)
# One DVE instruction per tile (8 ALU stages), runs concurrently with the
# ACT engine's exact tanh.  Registered at import time.
# --------------------------------------------------------------------------
from concourse.dve_spec import (
    Spec, Src0, C0, C1, C2, C3, Zero, maxx, minn, sq, _spill_c3_to_src1,
)
import concourse.dve_ops as _dve_ops_mod
from concourse.dve_ops import DveOp

TANH_B = 2.1350599
TANH_A = 0.94666379
TANH_CB = -0.19501118
TANH_CC = 0.01945195


def _tanh_pc_ref(in0, in1, s0, s1, imm2):
    prt = in0.shape[0]
    ucv = np.clip(
        in0.astype(np.float32),
        -np.asarray(s0, np.float32),
        np.asarray(s0, np.float32),
    )
    vv = ucv * ucv
    a = np.asarray(in1, np.float32).reshape(prt, -1)[:, :1]
    return ((vv * imm2 + s1) * vv + a) * ucv


_uc = maxx(minn(Src0, C0), Zero - C0)
_v = sq(_uc)
_body = _spill_c3_to_src1(((_v * C2 + C1) * _v + C3) * _uc)
TANH_PC = DveOp(
    "TANH_PC",
    Spec(body=_body, reference=_tanh_pc_ref),
    subdim=False,
    uops_sha={"v3": "b46f8204b307e3bf", "v4": "e95adf23d01b2e24"},
)
if TANH_PC.name not in _dve_ops_mod._SUB_OPCODE_FOR_NAME:
    _dve_ops_mod.OPS.append(TANH_PC)
    _dve_ops_mod._SUB_OPCODE_FOR_NAME[TANH_PC.name] = (
        _dve_ops_mod._CUSTOM_DVE_ROW_BASE + len(_dve_ops_mod.OPS) - 1
    )
    _dve_ops_mod.CUSTOM_DVE_SPECS[TANH_PC.name] = TANH_PC.spec

F32 = mybir.dt.float32
BF16 = mybir.dt.bfloat16
F8 = mybir.dt.float8e4
AFT = mybir.ActivationFunctionType
DR = mybir.MatmulPerfMode.DoubleRow

N, K, P = 16384, 64, 64
H1, H2 = 256, 256
NCORES = 8
RPC = N // NCORES          # rows per core = 2048
NT = 512                   # rows per n-tile (free dim)
TILES = RPC // NT          # 4
NPAIR = K // 2             # 32 component pairs
NGRP = NPAIR // 2          # 16 groups of 2 pairs (4 components)
LOG2PI = float(np.log(2.0 * np.pi))
C_OFF = 115.0              # global exp offset; keeps exp(lv + C) in fp32 range

_cached = {}
TRACE = False          # set by test harness to capture an NTFF profile
LAST_RESULT = None     # BassKernelResults of the most recent run


def _build_program(use_b2: bool):
    nc = bacc.Bacc("TRN2", target_bir_lowering=False, debug=False)

    xT = nc.dram_tensor("xT", [P + 1, RPC], F32, kind="ExternalInput")
    xsqT = nc.dram_tensor("xsqT", [P, RPC], F32, kind="ExternalInput")
    A_all = nc.dram_tensor("A_all", [P + 1, K * H1], BF16, kind="ExternalInput")
    W2DR = nc.dram_tensor("W2DR", [128, 2, 256], F8, kind="ExternalInput")
    W3DR = nc.dram_tensor("W3DR", [128, 2, NPAIR * 256], F8, kind="ExternalInput")
    UV = nc.dram_tensor("UV", [P, 2 * K], F32, kind="ExternalInput")
    BEx = nc.dram_tensor("BEx", [K, 1], F32, kind="ExternalInput")
    B3R = nc.dram_tensor("B3R", [128, NPAIR], F32, kind="ExternalInput")
    B2h = nc.dram_tensor("B2h", [128, 2], F32, kind="ExternalInput")
    Sel = nc.dram_tensor("Sel", [128, 2], BF16, kind="ExternalInput")
    ones = nc.dram_tensor("ones", [K, 1], F32, kind="ExternalInput")
    acc_out = nc.dram_tensor("acc_out", [1, RPC], F32, kind="ExternalOutput")

    with tile.TileContext(nc) as tc:
        with (
            tc.tile_pool(name="const", bufs=1) as cpool,
            tc.tile_pool(name="io", bufs=3) as iop,
            tc.tile_pool(name="h1pool", bufs=4) as h1pool,
            tc.tile_pool(name="h2pool", bufs=4) as h2pool,
            tc.tile_pool(name="expl", bufs=4) as explp,
            tc.tile_pool(name="ed", bufs=2) as edp,
            tc.tile_pool(name="rv", bufs=3) as rvp,
            tc.tile_pool(name="tmp", bufs=2) as tmpp,
            tc.tile_pool(name="pmlp", bufs=3, space="PSUM") as pmlp,
            tc.tile_pool(name="plg", bufs=2, space="PSUM") as plg,
        ):
            # --- constants; small hot ones first so tile 0 can start ---
            UV_sb = cpool.tile([P, 2 * K], F32)
            nc.sync.dma_start(UV_sb[:], UV[:])
            BEx_sb = cpool.tile([K, 1], F32)
            nc.sync.dma_start(BEx_sb[:], BEx[:])
            A_sb = cpool.tile([P + 1, K * H1], BF16)
            ACH = K * H1 // 8
            W2_sb = cpool.tile([128, 2, 256], F8)
            W3_sb = cpool.tile([128, 2, NPAIR * 256], F8)
            B3r_sb = cpool.tile([128, NPAIR], F32)
            B2_sb = cpool.tile([128, 2], F32)
            ones_sb = cpool.tile([K, 1], F32)
            Sel_sb = cpool.tile([128, 2], BF16)
            aconst = cpool.tile([128, 1], F32)
            nc.vector.memset(aconst[:], TANH_A)

            gp = nc.gpsimd if hasattr(nc.gpsimd, "tensor_copy") else nc.vector

            def tanh_dve(dst, src):
                nc.vector._custom_dve(
                    TANH_PC, out=dst, in0=src, in1=aconst[:],
                    s0=TANH_B, s1=TANH_CB, imm2=TANH_CC,
                )

            def tanh_act(dst, src):
                nc.scalar.activation(dst, src, AFT.Tanh)

            NP = TILES * NPAIR          # 128 pairs, software-pipelined flat
            h1s_of, h2s_of, lg_of, ex_of = {}, {}, {}, {}
            tile_io = {}                # t -> (xt_bf, E_sb, SD_S, SD_D)
            pend_epi = [None]           # (E_sb, SD_S, SD_D, col)
            pend_acc = [None]           # (tt2, col)

            def prologue(t):
                """Input DMAs + cast + q matmuls + E exp for tile t."""
                col = slice(t * NT, (t + 1) * NT)
                xt = iop.tile([P + 1, NT], F32, tag="xt")
                nc.sync.dma_start(xt[:], xT[:, col])
                xs = iop.tile([P, NT], F32, tag="xs")
                nc.sync.dma_start(xs[:], xsqT[:, col])
                xt_bf = iop.tile([P + 1, NT], BF16, tag="xtb")
                gp.tensor_copy(xt_bf[:], xt[:])

                if t == 0:
                    def a_chunk(ch):
                        nc.sync.dma_start(
                            A_sb[:, ch * ACH : (ch + 1) * ACH],
                            A_all[:, ch * ACH : (ch + 1) * ACH],
                        )

                    WCH = NPAIR * 256 // 4
                    a_chunk(0)
                    nc.sync.dma_start(W2_sb[:], W2DR[:])
                    nc.sync.dma_start(W3_sb[:, :, 0:WCH], W3DR[:, :, 0:WCH])
                    nc.sync.dma_start(B3r_sb[:], B3R[:])
                    a_chunk(1)
                    nc.sync.dma_start(Sel_sb[:], Sel[:])
                    for ch in range(2, 8):
                        a_chunk(ch)
                        if ch <= 4:
                            w0 = (ch - 1) * WCH
                            nc.sync.dma_start(
                                W3_sb[:, :, w0 : w0 + WCH],
                                W3DR[:, :, w0 : w0 + WCH],
                            )
                    nc.sync.dma_start(B2_sb[:], B2h[:])
                    nc.sync.dma_start(ones_sb[:], ones[:])

                # E = exp(-0.5*(U.x^2 + V.x) + B_k); q borrows an lg-ring bank
                q_ps = plg.tile([128, NT], F32, tag="lg")
                nc.tensor.matmul(
                    q_ps[0:K, :], UV_sb[:, 0:K], xs[:], start=True, stop=False
                )
                nc.tensor.matmul(
                    q_ps[0:K, :], UV_sb[:, K : 2 * K], xt[0:P, :],
                    start=False, stop=True,
                )
                E_sb = edp.tile([K, NT], F32, tag="E")
                nc.scalar.activation(
                    E_sb[:], q_ps[0:K, :], AFT.Exp, bias=BEx_sb[:], scale=-0.5
                )
                SD_S = edp.tile([K, NT], F32, tag="SDs")
                SD_D = edp.tile([K, NT], BF16, tag="SDd")
                tile_io[t] = (xt_bf, E_sb, SD_S, SD_D)
                return col

            col_of = {0: prologue(0)}

            def stage_mm1_tanh1(p):
                """mm1 (bf16, augmented bias row) + tanh1 for both comps."""
                xt_bf = tile_io[p // NPAIR][0]
                for par in range(2):
                    k = 2 * (p % NPAIR) + par
                    h1p = pmlp.tile([128, 2, NT], F32, tag="mlp")
                    for half in range(2):
                        nc.tensor.matmul(
                            h1p[:, half, :],
                            A_sb[:, k * H1 + half * 128 : k * H1 + (half + 1) * 128],
                            xt_bf[:],
                            start=True,
                            stop=True,
                        )
                    h1s = h1pool.tile([128, 2, NT], F8, tag="h1s")
                    if par == 0:
                        tanh_act(h1s[:, :, :], h1p[:, :, :])
                    else:
                        tanh_dve(h1s[:, :, :], h1p[:, :, :])
                    h1s_of[(p, par)] = h1s

            def stage_mm2_tanh2(p):
                """fp8 DoubleRow mm2 + tanh2 for both comps."""
                for par in range(2):
                    h1s = h1s_of.pop((p, par))
                    h2p = pmlp.tile([128, 2, NT], F32, tag="mlp")
                    for v in range(2):
                        nc.tensor.matmul(
                            h2p[:, v, :],
                            W2_sb[:, :, v * 128 : (v + 1) * 128],
                            h1s[:, :, :],
                            start=True,
                            stop=True,
                            perf_mode=DR,
                        )
                    h2s = h2pool.tile([128, 2, NT], F8, tag="h2s")
                    if use_b2:
                        for v in range(2):
                            nc.scalar.activation(
                                h2s[:, v, :], h2p[:, v, :], AFT.Tanh,
                                bias=B2_sb[:, v : v + 1],
                            )
                    elif par == 0:
                        tanh_dve(h2s[:, :, :], h2p[:, :, :])
                    else:
                        tanh_act(h2s[:, :, :], h2p[:, :, :])
                    h2s_of[(p, par)] = h2s

            def stage_mm3_exp(p):
                """fp8 DR logits + exp.  W3 is rotated per component so the
                softmax diagonal lands at partition 0 (even) / 64 (odd), and
                padded per parity so the pair accumulates into one full-height
                PSUM bank (DR requires out base partition 0)."""
                pg = p % NPAIR
                lg2 = plg.tile([128, NT], F32, tag="lg")
                nc.tensor.matmul(
                    lg2[:], W3_sb[:, :, pg * 256 : pg * 256 + 128],
                    h2s_of.pop((p, 0))[:, :, :],
                    start=True, stop=False, perf_mode=DR,
                )
                nc.tensor.matmul(
                    lg2[:], W3_sb[:, :, pg * 256 + 128 : pg * 256 + 256],
                    h2s_of.pop((p, 1))[:, :, :],
                    start=False, stop=True, perf_mode=DR,
                )
                ex2 = explp.tile([128, NT], BF16, tag="ex")
                nc.scalar.activation(
                    ex2[:], lg2[:], AFT.Exp, bias=B3r_sb[:, pg : pg + 1]
                )
                sdd = tile_io[p // NPAIR][3]
                nc.sync.dma_start(sdd[2 * pg : 2 * pg + 1, :], ex2[0:1, :])
                nc.sync.dma_start(sdd[2 * pg + 1 : 2 * pg + 2, :], ex2[64:65, :])
                lg_of[p] = lg2
                ex_of[p] = ex2

            def stage_sel(p):
                """S selector matmul reuses the pair's own lg2 bank (rows 0:2,
                idle after exp); per-pair fast reciprocal off PSUM, then DMA
                places the two 1/S rows into sinv64."""
                ex2 = ex_of.pop(p)
                lg2 = lg_of.pop(p)
                pg = p % NPAIR
                nc.tensor.matmul(
                    lg2[0:2, :], Sel_sb[:], ex2[:], start=True, stop=True
                )
                rinv = rvp.tile([2, NT], F32, tag="rv")
                nc.vector.reciprocal_approx_fast(rinv[:], lg2[0:2, :])
                sinv = tile_io[p // NPAIR][2]
                nc.sync.dma_start(sinv[2 * pg : 2 * pg + 2, :], rinv[:])

            def flush_epi_dve():
                if pend_epi[0] is None:
                    return
                E_prev, sinv, sdd, pcol = pend_epi[0]
                pend_epi[0] = None
                tt = tmpp.tile([K, NT], F32, tag="tmp2")
                gp.tensor_mul(tt[:], sdd[:], sinv[:])
                tt2 = tmpp.tile([K, NT], F32, tag="tt2")
                gp.tensor_mul(tt2[:], tt[:], E_prev[:])
                pend_acc[0] = (tt2, pend_col[0])

            def flush_acc():
                if pend_acc[0] is None:
                    return
                tt2, acc_col = pend_acc[0]
                pend_acc[0] = None
                acc_ps = plg.tile([128, NT], F32, tag="lg")
                nc.tensor.matmul(
                    acc_ps[0:1, :], ones_sb[:], tt2[:], start=True, stop=True
                )
                acc_sb = tmpp.tile([1, NT], F32, tag="acc")
                nc.vector.tensor_copy(acc_sb[:], acc_ps[0:1, :])
                nc.sync.dma_start(acc_out[0:1, acc_col], acc_sb[:])

            pend_col = [None]

            for it in range(NP + 3):
                if 1 <= it <= NP:
                    stage_mm2_tanh2(it - 1)
                if it >= 3:
                    stage_sel(it - 3)
                if it % NPAIR == 16:
                    flush_acc()
                if it < NP:
                    stage_mm1_tanh1(it)
                if 2 <= it <= NP + 1:
                    stage_mm3_exp(it - 2)
                if it < NP and it % NPAIR == NPAIR - 8:
                    t = it // NPAIR
                    if t + 1 < TILES:
                        col_of[t + 1] = prologue(t + 1)
                if it % NPAIR == NPAIR - 1:
                    t = it // NPAIR
                    _, E_sb, SD_S, SD_D = tile_io[t]
                    pend_epi[0] = (E_sb, SD_S, SD_D, col_of[t])
                if it % NPAIR == 8 and it > NPAIR:
                    pend_col[0] = col_of[it // NPAIR - 1]
                    flush_epi_dve()

            # tail: epilogue for the last tile
            pend_col[0] = col_of[TILES - 1]
            flush_epi_dve()
            flush_acc()

    nc.finalize()
    return nc


def _prep_consts(m, log_s, W1, b1, W2, b2, W3, b3):
    import ml_dtypes

    bf16 = ml_dtypes.bfloat16
    f8 = ml_dtypes.float8_e4m3fn
    inv_s = np.exp(-np.asarray(log_s, np.float64))          # [K,P]
    m64 = np.asarray(m, np.float64)
    W1_64 = np.asarray(W1, np.float64)
    ims = inv_s * m64                                       # [K,P]

    # A_all[p, k*H1+h] = W1[h,p]*inv_s[k,p]; row P = c1_k[h]
    A = W1_64[None, :, :] * inv_s[:, None, :]               # [K,H1,P]
    A_all = np.empty((P + 1, K * H1), np.float32)
    A_all[:P] = A.transpose(2, 0, 1).reshape(P, K * H1)
    c1 = np.asarray(b1, np.float64)[None, :] - np.einsum("hp,kp->kh", W1_64, ims)
    A_all[P] = c1.reshape(K * H1).astype(np.float32)

    # DoubleRow fp8 weights: lhsT[p, j, o] = W[o, p + 128j]
    W2a = np.asarray(W2, np.float32)
    W2dr = np.empty((128, 2, 256), np.float32)
    for j in range(2):
        for v in range(2):
            W2dr[:, j, v * 128 : (v + 1) * 128] = W2a[
                v * 128 : (v + 1) * 128, 128 * j : 128 * (j + 1)
            ].T
    # W3 per-pair stationaries: rotated per component so comp k's diagonal
    # class lands at out partition 0 (even) / 64 (odd), padded to 128 out
    # columns per parity (DR requires out base partition 0; the pair
    # accumulates into one full-height PSUM bank).
    W3a = np.asarray(W3, np.float32)
    W3dr = np.zeros((128, 2, NPAIR * 256), np.float32)
    cidx = np.arange(64)
    for pg in range(NPAIR):
        for par in range(2):
            k = 2 * pg + par
            rot = W3a[(cidx + k) % 64, :]          # [c', 256]
            for j in range(2):
                base = pg * 256 + 192 * par
                W3dr[:, j, base : base + 64] = rot[:, 128 * j : 128 * (j + 1)].T

    UV = np.empty((P, 2 * K), np.float32)
    UV[:, 0:K] = (inv_s**2).T
    UV[:, K : 2 * K] = (-2.0 * m64 * inv_s**2).T

    w_k = np.sum(ims**2, axis=1)                            # [K]
    log_det = -np.asarray(log_s, np.float64).sum(axis=1)    # [K]
    BEx = (-0.5 * w_k - 0.5 * P * LOG2PI + log_det + C_OFF).astype(np.float32)

    b3a = np.asarray(b3, np.float32)
    B3R = np.empty((128, NPAIR), np.float32)
    for pg in range(NPAIR):
        B3R[0:64, pg] = b3a[(cidx + 2 * pg) % 64]
        B3R[64:128, pg] = b3a[(cidx + 2 * pg + 1) % 64]
    B2h = np.stack([np.asarray(b2)[:128], np.asarray(b2)[128:]], axis=1).astype(
        np.float32
    )

    Sel = np.zeros((128, 2), np.float32)
    Sel[0:64, 0] = 1.0
    Sel[64:128, 1] = 1.0

    return {
        "A_all": A_all.astype(bf16),
        "W2DR": W2dr.astype(f8),
        "W3DR": W3dr.astype(f8),
        "UV": UV,
        "BEx": BEx.reshape(K, 1),
        "B3R": B3R,
        "B2h": B2h,
        "Sel": Sel.astype(bf16),
        "ones": np.ones((K, 1), np.float32),
    }


def kernel(x, m, log_s, W1, b1, W2, b2, W3, b3):
    x = np.asarray(x, np.float32)
    consts = _prep_consts(m, log_s, W1, b1, W2, b2, W3, b3)
    use_b2 = bool(np.any(np.asarray(b2)))

    key = ("prog", use_b2)
    if key not in _cached:
        _cached[key] = _build_program(use_b2)
    nc = _cached[key]

    xT = np.empty((P + 1, N), np.float32)
    xT[:P] = x.T
    xT[P] = 1.0
    xsqT = (x.T.astype(np.float64) ** 2).astype(np.float32)

    in_maps = []
    for i in range(NCORES):
        col = slice(i * RPC, (i + 1) * RPC)
        im = {"xT": np.ascontiguousarray(xT[:, col]),
              "xsqT": np.ascontiguousarray(xsqT[:, col])}
        im.update(consts)
        in_maps.append(im)

    res = bass_utils.run_bass_kernel_spmd(
        nc, in_maps, list(range(NCORES)), trace=TRACE
    )
    global LAST_RESULT
    LAST_RESULT = res
    acc = np.concatenate([r["acc_out"].reshape(RPC) for r in res.results])
    return (np.log(acc.astype(np.float64)) - C_OFF).astype(np.float32)
